# revision 8
# baseline (speedup 1.0000x reference)
"""Trainium2 Bass kernel for the n-ary span-compose problem (gnn_message_passing).

Strategy v3 (zero cross-core communication, host-planned, no dma_gather):
  The host resolves the full version DAG (which value every compose reads and
  which write wins each output position).  Needed composes form tiny connected
  components, distributed over 8 cores balancing MLP work and embedding-stream
  length (with token-overlap-aware clustering to cut duplication).

  Each core keeps a TRANSPOSED value log resident in SBUF as two f32 planes:
      vT[j][p, s] = value_of_slot_s[j*128 + p]   (2 x [128, nslots] f32)
  slot space: 0 = zeros (pad reads), [1, 1+NCTOKP) = compose-read tokens,
  then compose outputs level by level, then base-final filler tokens (so
  compose gather bounds never cover fillers).

  Phase A: the per-core token stream is compacted ON HOST into a dense
  [NTOKP, 768] bf16 input and streamed with xbar transpose DMA
  (dma_start_transpose -> pre-transposed lhsT-ready tiles, no GpSimd
  descriptor generation).  Down-projection runs as a transposed GEMM
  (lhsT = w_down) writing straight into vT0/vT1.

  Compose tiles (width 256/128) fetch their 4 operands per compose per plane
  with ap_gather (GpSimd SIMD ucode gather from SBUF along the free dim),
  sum with 3 contiguous DVE adds per plane (last add writes the bf16
  transposed mean), fold the 1/cnt mean scale into the GELU's scale
  argument, and run both MLP layers as transposed GEMMs (lhsT = wc1 / wc2)
  so no PE transposes are needed anywhere.  Outputs are copied from PSUM
  straight into vT0/vT1.

  The logs are dumped to DRAM incrementally on the scalar engine (the sync
  engine does nothing but the xbar stream); the host assembles the final
  [16, 2048, 256] output from (core, slot) maps.  Filler A-chunks are
  emitted after the compose tiles so the PE stream has no gaps.
"""

import sys
import types
import numpy as np
import ml_dtypes
from contextlib import ExitStack

import concourse.bass as bass
import concourse.bacc as bacc
import concourse.mybir as mybir
import concourse.tile as tile
from concourse.bass_utils import run_bass_kernel_spmd

N_CORES = 8
NPOS = 16 * 2048
NLEV = 3
NSPAN = 4096
VOCAB = 32000
D = 768
CD = 256
HD = 1024
P = 128
F32 = mybir.dt.float32
BF16 = mybir.dt.bfloat16
I16 = mybir.dt.int16

ACHUNK = 256      # rows per phase-A stream chunk
WTILE = 256       # composes per supertile (last tile of a level may be 128)


# --------------------------------------------------------------------------
# host planner
# --------------------------------------------------------------------------

def _last_wins(tgt):
    u, first_rev = np.unique(tgt[::-1], return_index=True)
    return u, len(tgt) - 1 - first_rev


def plan(chunk_input_ids, spans_list):
    ids = np.asarray(chunk_input_ids).astype(np.int64).ravel()
    ids = np.where(ids == -100, 0, ids)
    assert ids.size == NPOS

    # ---- version DAG ----
    ver = np.arange(NPOS, dtype=np.int64)
    comp_reads, comp_cnt = [], []
    for l, spans in enumerate(spans_list):
        spans = np.asarray(spans).astype(np.int64)
        mask = spans != -100
        tgt = spans.max(-1) + 1
        idx = np.where(mask, spans, 0)
        rd = np.where(mask, ver[idx], -1)
        comp_reads.append(rd)
        comp_cnt.append(mask.sum(-1))
        u, win = _last_wins(tgt)
        ver[u] = NPOS + l * NSPAN + win
    final_ver = ver

    # ---- liveness ----
    needed = [np.zeros(NSPAN, bool) for _ in range(NLEV)]
    fin_comp = final_ver[final_ver >= NPOS] - NPOS
    for l in range(NLEV):
        needed[l][fin_comp[fin_comp // NSPAN == l] % NSPAN] = True
    for l in range(NLEV - 1, -1, -1):
        rd = comp_reads[l][needed[l]].ravel()
        rd = rd[rd >= NPOS] - NPOS
        for l2 in range(l):
            needed[l2][rd[rd // NSPAN == l2] % NSPAN] = True

    # ---- connected components over comp->comp read edges ----
    parent = {}

    def find(x):
        root = x
        while parent[root] != root:
            root = parent[root]
        while parent[x] != root:
            parent[x], x = root, parent[x]
        return root

    for l in range(NLEV):
        for r in np.nonzero(needed[l])[0]:
            parent[l * NSPAN + r] = l * NSPAN + r
    for l in range(NLEV):
        rows = np.nonzero(needed[l])[0]
        rd = comp_reads[l][rows]
        for i, r in enumerate(rows):
            for v in rd[i]:
                if v >= NPOS:
                    ra, rb = find(l * NSPAN + int(r)), find(int(v - NPOS))
                    if ra != rb:
                        parent[ra] = rb

    comps_by_root = {}
    for node in parent:
        comps_by_root.setdefault(find(node), []).append(node)

    # ---- group metadata: per-level comp counts + compose-read token sets ----
    groups = []
    for g in comps_by_root.values():
        per_lvl = np.zeros(NLEV, np.int64)
        toks = set()
        for uid in g:
            l = uid // NSPAN
            per_lvl[l] += 1
            for v in comp_reads[l][uid % NSPAN]:
                v = int(v)
                if 0 <= v < NPOS:
                    toks.add(int(ids[v]))
        groups.append((g, per_lvl, toks))

    # ---- greedy assignment: balance MLP comps + token stream, cluster by
    #      token overlap (newtok term) ----
    WC, WT = 18.5, 7.0   # ~ns per compose (MLP) / per streamed token row
    comp_core = {}
    compload = np.zeros((N_CORES, NLEV))
    tokload = np.zeros(N_CORES)
    tok_sets = [set() for _ in range(N_CORES)]
    order = sorted(range(len(groups)),
                   key=lambda i: -(len(groups[i][0]) * 4 + len(groups[i][2])))
    for gi in order:
        g, per_lvl, toks = groups[gi]
        best, bestc = None, 0
        for c in range(N_CORES):
            newtok = sum(1 for t in toks if t not in tok_sets[c])
            score = (WC * (compload[c].sum() + per_lvl.sum())
                     + WT * (tokload[c] + newtok)
                     + 0.25 * WC * (compload[c] + per_lvl).max())
            if best is None or score < best:
                best, bestc = score, c
        c = bestc
        for uid in g:
            comp_core[uid] = c
        compload[c] += per_lvl
        tokload[c] += sum(1 for t in toks if t not in tok_sets[c])
        tok_sets[c].update(toks)

    # ---- base-final tokens: canonical core (prefer one that has it) ----
    is_comp_final = final_ver >= NPOS
    base_pos = np.nonzero(~is_comp_final)[0]
    tok_canon = {}
    filler = [[] for _ in range(N_CORES)]
    fill_load = np.zeros(N_CORES, np.int64)
    for p in base_pos:
        t = int(ids[p])
        if t in tok_canon:
            continue
        for c in range(N_CORES):
            if t in tok_sets[c]:
                tok_canon[t] = c
                break
        else:
            c = int(np.argmin(fill_load))
            tok_canon[t] = c
            filler[c].append(t)
            fill_load[c] += 1

    # ---- per-core streams / slots / tiles ----
    def rup(x, m):
        return -(-int(x) // m) * m

    core_ctok = []     # compose-read tokens in first-use order
    for c in range(N_CORES):
        lst, seen = [], set()
        for l in range(NLEV):
            rows = sorted(uid % NSPAN for uid, cc in comp_core.items()
                          if cc == c and uid // NSPAN == l)
            for r in rows:
                for v in comp_reads[l][r]:
                    v = int(v)
                    if 0 <= v < NPOS:
                        t = int(ids[v])
                        if t not in seen:
                            seen.add(t)
                            lst.append(t)
        core_ctok.append(lst)

    NCTOKP = rup(max(len(l) for l in core_ctok), ACHUNK)
    FILLP = rup(max(len(f) for f in filler), ACHUNK)
    A1_CHUNKS = NCTOKP // ACHUNK
    A_CHUNKS = A1_CHUNKS + FILLP // ACHUNK
    NTOKP = A_CHUNKS * ACHUNK

    ncmp = np.zeros((N_CORES, NLEV), np.int64)
    for uid, c in comp_core.items():
        ncmp[c, uid // NSPAN] += 1
    NC = [int(rup(ncmp[:, l].max(), P)) for l in range(NLEV)]
    lvl_base = []
    b = 1 + NCTOKP
    for l in range(NLEV):
        lvl_base.append(b)
        b += NC[l]
    fill_base = b
    nslots = b + FILLP
    assert nslots < 32768

    # tile widths per level (shared across cores)
    tiles = []   # list of (level, base_slot, W)
    for l in range(NLEV):
        off = 0
        while off < NC[l]:
            w = WTILE if NC[l] - off >= WTILE else P
            tiles.append((l, lvl_base[l] + off, w))
            off += w

    inv_vals = set()
    core_rd = []
    core_bounds = []
    core_slot_of_comp = []
    core_tok_slot = []
    for c in range(N_CORES):
        slot_of_tok = {t: 1 + i for i, t in enumerate(core_ctok[c])}
        for i, t in enumerate(filler[c]):
            slot_of_tok[t] = fill_base + i
        core_tok_slot.append(slot_of_tok)
        slot_of_comp = {}
        rd_all = []
        bounds = []

        def vslot(v):
            v = int(v)
            if v == -1:
                return 0
            if v < NPOS:
                return slot_of_tok[int(ids[v])]
            return slot_of_comp[v - NPOS]

        for l in range(NLEV):
            rows = sorted(uid % NSPAN for uid, cc in comp_core.items()
                          if cc == c and uid // NSPAN == l)

            def row_bound(r):
                return max((vslot(v) for v in comp_reads[l][r]), default=0)
            rows = sorted(rows, key=lambda r: (row_bound(r), r))
            for i, r in enumerate(rows):
                slot_of_comp[l * NSPAN + int(r)] = lvl_base[l] + i
                inv_vals.add(1.0 / max(int(comp_cnt[l][r]), 1))
            rs = np.zeros((NC[l], 4), np.int64)
            for i, r in enumerate(rows):
                for k in range(4):
                    rs[i, k] = vslot(comp_reads[l][r, k])
            off = 0
            for (tl, tbase, w) in tiles:
                if tl != l:
                    continue
                blk = rs[off:off + w]          # [w, 4]
                rd_all.append(blk.T.reshape(-1))   # k-major [4*w]
                bounds.append(max(1, int(blk.max()) + 1))
                off += w
        core_rd.append(np.concatenate(rd_all))
        core_bounds.append(bounds)
        core_slot_of_comp.append(slot_of_comp)

    bounds = tuple(max(core_bounds[c][i] for c in range(N_CORES))
                   for i in range(len(tiles)))
    for i, (_, tbase, w) in enumerate(tiles):
        assert bounds[i] <= tbase

    if not inv_vals:
        inv_vals = {0.25}
    assert len(inv_vals) == 1, f"non-uniform span counts {inv_vals}"
    inv_uniform = float(inv_vals.pop())

    # ---- output assembly maps ----
    pos_core = np.empty(NPOS, np.int64)
    pos_slot = np.empty(NPOS, np.int64)
    for p in range(NPOS):
        v = int(final_ver[p])
        if v < NPOS:
            t = int(ids[v])
            c = tok_canon[t]
            pos_core[p] = c
            pos_slot[p] = core_tok_slot[c][t]
        else:
            c = comp_core[v - NPOS]
            pos_core[p] = c
            pos_slot[p] = core_slot_of_comp[c][v - NPOS]

    cores = []
    for c in range(N_CORES):
        cores.append(dict(ctok=core_ctok[c], fill=filler[c], rd=core_rd[c]))
    meta = dict(NTOKP=NTOKP, NCTOKP=NCTOKP, A1_CHUNKS=A1_CHUNKS,
                A_CHUNKS=A_CHUNKS, fill_base=fill_base,
                tiles=tuple(tiles), bounds=bounds, nslots=nslots,
                inv=inv_uniform, pos_core=pos_core, pos_slot=pos_slot)
    return cores, meta


def wrap_idx16(idx):
    """[n] -> [128, n/16] int16 layout for gpsimd gathers (i -> (i%16, i//16))."""
    idx = np.asarray(idx, np.int64)
    n = len(idx)
    assert n % 16 == 0 and idx.max() < 32768 and idx.min() >= 0
    w = idx.reshape(n // 16, 16).T.astype(np.int16)
    return np.tile(w, (8, 1))


# --------------------------------------------------------------------------
# bass program
# --------------------------------------------------------------------------

def build_bass(NTOKP, NCTOKP, A1_CHUNKS, A_CHUNKS, fill_base, tiles, bounds,
               nslots, has_bd, has_b1, has_b2, inv):
    nc = bacc.Bacc("TRN2", target_bir_lowering=False, debug=False,
                   num_devices=N_CORES, num_swdge_queues=4)

    emb_s = nc.dram_tensor("emb_s", [NTOKP, D], BF16, kind="ExternalInput")
    w_down = nc.dram_tensor("w_down", [D, CD], BF16, kind="ExternalInput")
    b_down = nc.dram_tensor("b_down", [1, CD], F32, kind="ExternalInput")
    wc1 = nc.dram_tensor("wc1", [CD, HD], BF16, kind="ExternalInput")
    bc1 = nc.dram_tensor("bc1", [1, HD], F32, kind="ExternalInput")
    wc2 = nc.dram_tensor("wc2", [HD, CD], BF16, kind="ExternalInput")
    bc2 = nc.dram_tensor("bc2", [1, CD], F32, kind="ExternalInput")
    tot_idx = sum(4 * w for (_, _, w) in tiles)
    rd_idx = nc.dram_tensor("rd_idx", [P, tot_idx // 16], I16,
                            kind="ExternalInput")
    vlogT = nc.dram_tensor("vlogT", [P, nslots, 2], BF16,
                           kind="ExternalOutput")

    with tile.TileContext(nc) as tc, ExitStack() as ctx:
        cst = ctx.enter_context(tc.tile_pool(name="cst", bufs=1))
        sb = ctx.enter_context(tc.tile_pool(name="sb", bufs=3))
        ps = ctx.enter_context(tc.tile_pool(name="ps", bufs=2, space="PSUM"))

        rd_sb = cst.tile([P, tot_idx // 16], I16)
        nc.scalar.dma_start(rd_sb[:], rd_idx[:])

        # weights as lhsT chunks, pre-split into contiguous 128-wide m-tiles
        w_sb = cst.tile([P, D // P, CD // P, P], BF16)
        for k in range(D // P):
            for j in range(CD // P):
                nc.scalar.dma_start(
                    w_sb[:, k, j, :],
                    w_down[k * P:(k + 1) * P, j * P:(j + 1) * P])
        wc1_sb = cst.tile([P, CD // P, HD // P, P], BF16)
        for k in range(CD // P):
            for i in range(HD // P):
                nc.scalar.dma_start(
                    wc1_sb[:, k, i, :],
                    wc1[k * P:(k + 1) * P, i * P:(i + 1) * P])
        wc2_sb = cst.tile([P, HD // P, CD // P, P], BF16)
        for k in range(HD // P):
            for j in range(CD // P):
                nc.scalar.dma_start(
                    wc2_sb[:, k, j, :],
                    wc2[k * P:(k + 1) * P, j * P:(j + 1) * P])

        ones1 = cst.tile([1, WTILE], F32)
        nc.vector.memset(ones1[:], 1.0)
        bd_sb = cst.tile([1, CD], F32)
        nc.scalar.dma_start(bd_sb[:], b_down[:])
        bc1_sb = cst.tile([1, HD], F32)
        nc.scalar.dma_start(bc1_sb[:], bc1[:])
        bc2_sb = cst.tile([1, CD], F32)
        nc.scalar.dma_start(bc2_sb[:], bc2[:])

        # the SBUF-resident transposed value log, f32, one plane per cd half,
        # plus a bf16 mirror used only for dumping to DRAM
        vT = [cst.tile([P, nslots], F32, name=f"vT{j}") for j in range(2)]
        vTb = cst.tile([P, nslots, 2], BF16)
        for j in range(2):
            nc.vector.memset(vT[j][:, 0:1], 0.0)
        nc.vector.memset(vTb[:, 0:1, :], 0.0)
        nc.scalar.dma_start(vlogT[:, 0:1, :], vTb[:, 0:1, :])

        def a_chunk(ci, s0):
            """stream + down-project rows [ci*ACHUNK, ...) into slots [s0...)"""
            embT = sb.tile([P, D // P, ACHUNK], BF16, tag="embT", bufs=6)
            nc.sync.dma_start_transpose(
                embT[:], emb_s[ci * ACHUNK:(ci + 1) * ACHUNK, :])
            for j in range(2):
                acc = ps.tile([P, ACHUNK], F32, tag="acc", bufs=3)
                if has_bd:
                    nc.tensor.matmul(acc[:], lhsT=bd_sb[:, j * P:(j + 1) * P],
                                     rhs=ones1[:, 0:ACHUNK],
                                     start=True, stop=False)
                for k in range(D // P):
                    nc.tensor.matmul(acc[:], lhsT=w_sb[:, k, j, :],
                                     rhs=embT[:, k, :],
                                     start=(k == 0 and not has_bd),
                                     stop=(k == D // P - 1))
                nc.vector.tensor_copy(out=vT[j][:, s0:s0 + ACHUNK], in_=acc[:])
                nc.scalar.copy(out=vTb[:, s0:s0 + ACHUNK, j], in_=acc[:])
            eng = nc.sync if ci % 2 == 0 else nc.scalar
            eng.dma_start(vlogT[:, s0:s0 + ACHUNK, :],
                          vTb[:, s0:s0 + ACHUNK, :])

        # ---- phase A1: compose-read token chunks ----
        for i in range(A1_CHUNKS):
            a_chunk(i, 1 + i * ACHUNK)

        # ---- compose supertiles ----
        idx_off = 0
        for ti, (l, tbase, w) in enumerate(tiles):
            bound = bounds[ti]
            idxs = rd_sb[:, idx_off:idx_off + 4 * w // 16]
            idx_off += 4 * w // 16
            meanT = sb.tile([P, 2, w], BF16, tag=f"meanT{w}")
            for j in range(2):
                g = sb.tile([P, 4 * w], F32, tag=f"g{w}_{j}", bufs=2)
                nc.gpsimd.ap_gather(
                    g[:].unsqueeze(2), vT[j][:, 0:bound].unsqueeze(2),
                    idxs, channels=P, num_elems=bound, d=1, num_idxs=4 * w)
                s01 = sb.tile([P, w], F32, tag=f"s01_{w}_{j}")
                nc.vector.tensor_add(out=s01[:], in0=g[:, 0 * w:1 * w],
                                     in1=g[:, 1 * w:2 * w])
                s23 = sb.tile([P, w], F32, tag=f"s23_{w}_{j}")
                nc.vector.tensor_add(out=s23[:], in0=g[:, 2 * w:3 * w],
                                     in1=g[:, 3 * w:4 * w])
                nc.vector.tensor_add(out=meanT[:, j, :], in0=s01[:], in1=s23[:])

            hT = sb.tile([P, HD // P, w], BF16, tag=f"hT{w}", bufs=2)
            for i in range(HD // P):
                phb = ps.tile([P, WTILE], F32, tag="ph", bufs=2)
                ph = phb[:, 0:w]
                if has_b1:
                    nc.tensor.matmul(ph, lhsT=bc1_sb[:, i * P:(i + 1) * P],
                                     rhs=ones1[:, 0:w], start=True, stop=False)
                for k in range(CD // P):
                    nc.tensor.matmul(ph, lhsT=wc1_sb[:, k, i, :],
                                     rhs=meanT[:, k, :],
                                     start=(k == 0 and not has_b1),
                                     stop=(k == CD // P - 1))
                nc.scalar.activation(
                    out=hT[:, i, :], in_=ph,
                    func=mybir.ActivationFunctionType.Gelu_apprx_tanh,
                    scale=float(inv))
            for j in range(2):
                pob = ps.tile([P, WTILE], F32, tag="po", bufs=2)
                po = pob[:, 0:w]
                if has_b2:
                    nc.tensor.matmul(po, lhsT=bc2_sb[:, j * P:(j + 1) * P],
                                     rhs=ones1[:, 0:w], start=True, stop=False)
                for k in range(HD // P):
                    nc.tensor.matmul(po, lhsT=wc2_sb[:, k, j, :],
                                     rhs=hT[:, k, :],
                                     start=(k == 0 and not has_b2),
                                     stop=(k == HD // P - 1))
                nc.vector.tensor_copy(out=vT[j][:, tbase:tbase + w], in_=po)
                nc.scalar.copy(out=vTb[:, tbase:tbase + w, j], in_=po)
            eng = nc.sync if ti % 2 == 0 else nc.scalar
            eng.dma_start(vlogT[:, tbase:tbase + w, :],
                          vTb[:, tbase:tbase + w, :])

        # ---- phase A2: filler token chunks ----
        for i in range(A1_CHUNKS, A_CHUNKS):
            a_chunk(i, fill_base + (i - A1_CHUNKS) * ACHUNK)

    nc.compile()
    return nc


_CACHE = {}


def _get_bass(key):
    if key not in _CACHE:
        _CACHE[key] = build_bass(*key)
    return _CACHE[key]


def _install_ntff_hook():
    try:
        import antenv.axon_hooks  # noqa: F401
        return
    except ImportError:
        pass
    try:
        import trn_agent_boot.trn_boot as _tb
        hooks = types.ModuleType('antenv.axon_hooks')
        hook = _tb._ntff_profile_via_ctypes('/opt/axon/libaxon_pjrt.so')
        hooks.get_axon_ntff_profile_hook = lambda: hook
        hooks.set_axon_ntff_profile_hook = lambda h: None
        sys.modules['antenv.axon_hooks'] = hooks
    except Exception:
        pass


def run(inputs, trace=False):
    """Returns (full_output, exec_time_ns or None)."""
    inp = {k: (np.asarray(v) if hasattr(v, 'shape') else v)
           for k, v in inputs.items()}
    spans_list = [inp["spans0"], inp["spans1"], inp["spans2"]]
    cores, meta = plan(inp["chunk_input_ids"], spans_list)

    def f32(x):
        return np.ascontiguousarray(x, np.float32)

    b_down = f32(inp["b_down"]).reshape(1, CD)
    bc1 = f32(inp["bc1"]).reshape(1, HD)
    bc2 = f32(inp["bc2"]).reshape(1, CD)
    has_bd = bool(np.any(b_down))
    has_b1 = bool(np.any(bc1))
    has_b2 = bool(np.any(bc2))

    nc = _get_bass((meta["NTOKP"], meta["NCTOKP"], meta["A1_CHUNKS"],
                    meta["A_CHUNKS"], meta["fill_base"], meta["tiles"],
                    meta["bounds"], meta["nslots"],
                    has_bd, has_b1, has_b2, meta["inv"]))

    emb_bf = np.asarray(inp["emb_table"], np.float32).astype(ml_dtypes.bfloat16)

    def bf16(x):
        return np.ascontiguousarray(
            np.asarray(x, np.float32).astype(ml_dtypes.bfloat16))

    shared = dict(
        w_down=bf16(inp["w_down"]),
        b_down=b_down,
        wc1=bf16(inp["wc1"]),
        bc1=bc1,
        wc2=bf16(inp["wc2"]),
        bc2=bc2,
    )
    NTOKP, NCTOKP = meta["NTOKP"], meta["NCTOKP"]
    in_maps = []
    for c in range(N_CORES):
        core = cores[c]
        m = dict(shared)
        stream = np.zeros((NTOKP, D), ml_dtypes.bfloat16)
        ct, fl = core["ctok"], core["fill"]
        if len(ct):
            stream[:len(ct)] = emb_bf[np.asarray(ct, np.int64)]
        if len(fl):
            stream[NCTOKP:NCTOKP + len(fl)] = emb_bf[np.asarray(fl, np.int64)]
        m["emb_s"] = stream
        m["rd_idx"] = wrap_idx16(core["rd"])
        in_maps.append(m)

    _install_ntff_hook()
    res = run_bass_kernel_spmd(nc, in_maps, core_ids=list(range(N_CORES)),
                               trace=trace)
    vals = []
    for c in range(N_CORES):
        d = np.asarray(res.results[c]["vlogT"]).astype(np.float32)
        vals.append(d.transpose(1, 2, 0).reshape(-1, CD))   # [nslots, 256]
    vals = np.stack(vals)                           # [8, nslots, 256]
    full = vals[meta["pos_core"], meta["pos_slot"]]
    return full.reshape(16, 2048, CD), res.exec_time_ns


def kernel(**inputs):
    out, _ = run(inputs, trace=False)
    return out


# revision 10
# speedup vs baseline: 1.0092x; 1.0092x over previous
"""Trainium2 Bass kernel for the n-ary span-compose problem (gnn_message_passing).

Strategy v3 (zero cross-core communication, host-planned, no dma_gather):
  The host resolves the full version DAG (which value every compose reads and
  which write wins each output position).  Needed composes form tiny connected
  components, distributed over 8 cores balancing MLP work and embedding-stream
  length (with token-overlap-aware clustering to cut duplication).

  Each core keeps a TRANSPOSED value log resident in SBUF as two f32 planes:
      vT[j][p, s] = value_of_slot_s[j*128 + p]   (2 x [128, nslots] f32)
  slot space: 0 = zeros (pad reads), [1, 1+NCTOKP) = compose-read tokens,
  then compose outputs level by level, then base-final filler tokens (so
  compose gather bounds never cover fillers).

  Phase A: the per-core token stream is compacted ON HOST into a dense
  [NTOKP, 768] bf16 input and streamed with xbar transpose DMA
  (dma_start_transpose -> pre-transposed lhsT-ready tiles, no GpSimd
  descriptor generation).  Down-projection runs as a transposed GEMM
  (lhsT = w_down) writing straight into vT0/vT1.

  Compose tiles (width 256/128) fetch their 4 operands per compose per plane
  with ap_gather (GpSimd SIMD ucode gather from SBUF along the free dim),
  sum with 3 contiguous DVE adds per plane (last add writes the bf16
  transposed mean), fold the 1/cnt mean scale into the GELU's scale
  argument, and run both MLP layers as transposed GEMMs (lhsT = wc1 / wc2)
  so no PE transposes are needed anywhere.  Outputs are copied from PSUM
  straight into vT0/vT1.

  The logs are dumped to DRAM incrementally on the scalar engine (the sync
  engine does nothing but the xbar stream); the host assembles the final
  [16, 2048, 256] output from (core, slot) maps.  Filler A-chunks are
  emitted after the compose tiles so the PE stream has no gaps.
"""

import sys
import types
import numpy as np
import ml_dtypes
from contextlib import ExitStack

import concourse.bass as bass
import concourse.bacc as bacc
import concourse.mybir as mybir
import concourse.tile as tile
from concourse.bass_utils import run_bass_kernel_spmd

N_CORES = 8
NPOS = 16 * 2048
NLEV = 3
NSPAN = 4096
VOCAB = 32000
D = 768
CD = 256
HD = 1024
P = 128
F32 = mybir.dt.float32
BF16 = mybir.dt.bfloat16
I16 = mybir.dt.int16

ACHUNK = 256      # rows per phase-A stream chunk
WTILE = 256       # composes per supertile (last tile of a level may be 128)


# --------------------------------------------------------------------------
# host planner
# --------------------------------------------------------------------------

def _last_wins(tgt):
    u, first_rev = np.unique(tgt[::-1], return_index=True)
    return u, len(tgt) - 1 - first_rev


def plan(chunk_input_ids, spans_list):
    ids = np.asarray(chunk_input_ids).astype(np.int64).ravel()
    ids = np.where(ids == -100, 0, ids)
    assert ids.size == NPOS

    # ---- version DAG ----
    ver = np.arange(NPOS, dtype=np.int64)
    comp_reads, comp_cnt = [], []
    for l, spans in enumerate(spans_list):
        spans = np.asarray(spans).astype(np.int64)
        mask = spans != -100
        tgt = spans.max(-1) + 1
        idx = np.where(mask, spans, 0)
        rd = np.where(mask, ver[idx], -1)
        comp_reads.append(rd)
        comp_cnt.append(mask.sum(-1))
        u, win = _last_wins(tgt)
        ver[u] = NPOS + l * NSPAN + win
    final_ver = ver

    # ---- liveness ----
    needed = [np.zeros(NSPAN, bool) for _ in range(NLEV)]
    fin_comp = final_ver[final_ver >= NPOS] - NPOS
    for l in range(NLEV):
        needed[l][fin_comp[fin_comp // NSPAN == l] % NSPAN] = True
    for l in range(NLEV - 1, -1, -1):
        rd = comp_reads[l][needed[l]].ravel()
        rd = rd[rd >= NPOS] - NPOS
        for l2 in range(l):
            needed[l2][rd[rd // NSPAN == l2] % NSPAN] = True

    # ---- connected components over comp->comp read edges ----
    parent = {}

    def find(x):
        root = x
        while parent[root] != root:
            root = parent[root]
        while parent[x] != root:
            parent[x], x = root, parent[x]
        return root

    for l in range(NLEV):
        for r in np.nonzero(needed[l])[0]:
            parent[l * NSPAN + r] = l * NSPAN + r
    for l in range(NLEV):
        rows = np.nonzero(needed[l])[0]
        rd = comp_reads[l][rows]
        for i, r in enumerate(rows):
            for v in rd[i]:
                if v >= NPOS:
                    ra, rb = find(l * NSPAN + int(r)), find(int(v - NPOS))
                    if ra != rb:
                        parent[ra] = rb

    comps_by_root = {}
    for node in parent:
        comps_by_root.setdefault(find(node), []).append(node)

    # ---- group metadata: per-level comp counts + compose-read token sets ----
    groups = []
    for g in comps_by_root.values():
        per_lvl = np.zeros(NLEV, np.int64)
        toks = set()
        for uid in g:
            l = uid // NSPAN
            per_lvl[l] += 1
            for v in comp_reads[l][uid % NSPAN]:
                v = int(v)
                if 0 <= v < NPOS:
                    toks.add(int(ids[v]))
        groups.append((g, per_lvl, toks))

    # ---- greedy assignment: balance MLP comps + token stream, cluster by
    #      token overlap (newtok term) ----
    WC, WT = 18.5, 7.0   # ~ns per compose (MLP) / per streamed token row
    comp_core = {}
    compload = np.zeros((N_CORES, NLEV))
    tokload = np.zeros(N_CORES)
    tok_sets = [set() for _ in range(N_CORES)]
    order = sorted(range(len(groups)),
                   key=lambda i: -(len(groups[i][0]) * 4 + len(groups[i][2])))
    for gi in order:
        g, per_lvl, toks = groups[gi]
        best, bestc = None, 0
        for c in range(N_CORES):
            newtok = sum(1 for t in toks if t not in tok_sets[c])
            score = (WC * (compload[c].sum() + per_lvl.sum())
                     + WT * (tokload[c] + newtok)
                     + 0.25 * WC * (compload[c] + per_lvl).max())
            if best is None or score < best:
                best, bestc = score, c
        c = bestc
        for uid in g:
            comp_core[uid] = c
        compload[c] += per_lvl
        tokload[c] += sum(1 for t in toks if t not in tok_sets[c])
        tok_sets[c].update(toks)

    # ---- base-final tokens: canonical core (prefer one that has it) ----
    is_comp_final = final_ver >= NPOS
    base_pos = np.nonzero(~is_comp_final)[0]
    tok_canon = {}
    filler = [[] for _ in range(N_CORES)]
    fill_load = np.zeros(N_CORES, np.int64)
    for p in base_pos:
        t = int(ids[p])
        if t in tok_canon:
            continue
        for c in range(N_CORES):
            if t in tok_sets[c]:
                tok_canon[t] = c
                break
        else:
            c = int(np.argmin(fill_load))
            tok_canon[t] = c
            filler[c].append(t)
            fill_load[c] += 1

    # ---- per-core streams / slots / tiles ----
    def rup(x, m):
        return -(-int(x) // m) * m

    core_ctok = []     # compose-read tokens in first-use order
    for c in range(N_CORES):
        lst, seen = [], set()
        for l in range(NLEV):
            rows = sorted(uid % NSPAN for uid, cc in comp_core.items()
                          if cc == c and uid // NSPAN == l)
            for r in rows:
                for v in comp_reads[l][r]:
                    v = int(v)
                    if 0 <= v < NPOS:
                        t = int(ids[v])
                        if t not in seen:
                            seen.add(t)
                            lst.append(t)
        core_ctok.append(lst)

    NCTOKP = rup(max(len(l) for l in core_ctok), ACHUNK)
    FILLP = rup(max(len(f) for f in filler), ACHUNK)
    A1_CHUNKS = NCTOKP // ACHUNK
    A_CHUNKS = A1_CHUNKS + FILLP // ACHUNK
    NTOKP = A_CHUNKS * ACHUNK

    ncmp = np.zeros((N_CORES, NLEV), np.int64)
    for uid, c in comp_core.items():
        ncmp[c, uid // NSPAN] += 1
    NC = [int(rup(ncmp[:, l].max(), P)) for l in range(NLEV)]
    lvl_base = []
    b = 1 + NCTOKP
    for l in range(NLEV):
        lvl_base.append(b)
        b += NC[l]
    fill_base = b
    nslots = b + FILLP
    assert nslots < 32768

    # tile widths per level (shared across cores)
    tiles = []   # list of (level, base_slot, W)
    for l in range(NLEV):
        off = 0
        while off < NC[l]:
            w = WTILE if NC[l] - off >= WTILE else P
            tiles.append((l, lvl_base[l] + off, w))
            off += w

    inv_vals = set()
    core_rd = []
    core_bounds = []
    core_slot_of_comp = []
    core_tok_slot = []
    for c in range(N_CORES):
        slot_of_tok = {t: 1 + i for i, t in enumerate(core_ctok[c])}
        for i, t in enumerate(filler[c]):
            slot_of_tok[t] = fill_base + i
        core_tok_slot.append(slot_of_tok)
        slot_of_comp = {}
        rd_all = []
        bounds = []

        def vslot(v):
            v = int(v)
            if v == -1:
                return 0
            if v < NPOS:
                return slot_of_tok[int(ids[v])]
            return slot_of_comp[v - NPOS]

        for l in range(NLEV):
            rows = sorted(uid % NSPAN for uid, cc in comp_core.items()
                          if cc == c and uid // NSPAN == l)

            def row_bound(r):
                return max((vslot(v) for v in comp_reads[l][r]), default=0)
            rows = sorted(rows, key=lambda r: (row_bound(r), r))
            for i, r in enumerate(rows):
                slot_of_comp[l * NSPAN + int(r)] = lvl_base[l] + i
                inv_vals.add(1.0 / max(int(comp_cnt[l][r]), 1))
            rs = np.zeros((NC[l], 4), np.int64)
            for i, r in enumerate(rows):
                for k in range(4):
                    rs[i, k] = vslot(comp_reads[l][r, k])
            off = 0
            for (tl, tbase, w) in tiles:
                if tl != l:
                    continue
                blk = rs[off:off + w]          # [w, 4]
                rd_all.append(blk.T.reshape(-1))   # k-major [4*w]
                bounds.append(max(1, int(blk.max()) + 1))
                off += w
        core_rd.append(np.concatenate(rd_all))
        core_bounds.append(bounds)
        core_slot_of_comp.append(slot_of_comp)

    bounds = tuple(max(core_bounds[c][i] for c in range(N_CORES))
                   for i in range(len(tiles)))
    for i, (_, tbase, w) in enumerate(tiles):
        assert bounds[i] <= tbase

    if not inv_vals:
        inv_vals = {0.25}
    assert len(inv_vals) == 1, f"non-uniform span counts {inv_vals}"
    inv_uniform = float(inv_vals.pop())

    # ---- output assembly maps ----
    pos_core = np.empty(NPOS, np.int64)
    pos_slot = np.empty(NPOS, np.int64)
    for p in range(NPOS):
        v = int(final_ver[p])
        if v < NPOS:
            t = int(ids[v])
            c = tok_canon[t]
            pos_core[p] = c
            pos_slot[p] = core_tok_slot[c][t]
        else:
            c = comp_core[v - NPOS]
            pos_core[p] = c
            pos_slot[p] = core_slot_of_comp[c][v - NPOS]

    cores = []
    for c in range(N_CORES):
        cores.append(dict(ctok=core_ctok[c], fill=filler[c], rd=core_rd[c]))
    meta = dict(NTOKP=NTOKP, NCTOKP=NCTOKP, A1_CHUNKS=A1_CHUNKS,
                A_CHUNKS=A_CHUNKS, fill_base=fill_base,
                tiles=tuple(tiles), bounds=bounds, nslots=nslots,
                inv=inv_uniform, pos_core=pos_core, pos_slot=pos_slot)
    return cores, meta


def wrap_idx16(idx):
    """[n] -> [128, n/16] int16 layout for gpsimd gathers (i -> (i%16, i//16))."""
    idx = np.asarray(idx, np.int64)
    n = len(idx)
    assert n % 16 == 0 and idx.max() < 32768 and idx.min() >= 0
    w = idx.reshape(n // 16, 16).T.astype(np.int16)
    return np.tile(w, (8, 1))


# --------------------------------------------------------------------------
# bass program
# --------------------------------------------------------------------------

def build_bass(NTOKP, NCTOKP, A1_CHUNKS, A_CHUNKS, fill_base, tiles, bounds,
               nslots, has_bd, has_b1, has_b2, inv):
    nc = bacc.Bacc("TRN2", target_bir_lowering=False, debug=False,
                   num_devices=N_CORES, num_swdge_queues=4)

    emb_s = nc.dram_tensor("emb_s", [NTOKP, D], BF16, kind="ExternalInput")
    w_down = nc.dram_tensor("w_down", [D, CD], BF16, kind="ExternalInput")
    b_down = nc.dram_tensor("b_down", [1, CD], F32, kind="ExternalInput")
    wc1 = nc.dram_tensor("wc1", [CD, HD], BF16, kind="ExternalInput")
    bc1 = nc.dram_tensor("bc1", [1, HD], F32, kind="ExternalInput")
    wc2 = nc.dram_tensor("wc2", [HD, CD], BF16, kind="ExternalInput")
    bc2 = nc.dram_tensor("bc2", [1, CD], F32, kind="ExternalInput")
    tot_idx = sum(4 * w for (_, _, w) in tiles)
    rd_idx = nc.dram_tensor("rd_idx", [P, tot_idx // 16], I16,
                            kind="ExternalInput")
    vlogT = nc.dram_tensor("vlogT", [P, nslots, 2], BF16,
                           kind="ExternalOutput")

    with tile.TileContext(nc) as tc, ExitStack() as ctx:
        cst = ctx.enter_context(tc.tile_pool(name="cst", bufs=1))
        sb = ctx.enter_context(tc.tile_pool(name="sb", bufs=3))
        ps = ctx.enter_context(tc.tile_pool(name="ps", bufs=2, space="PSUM"))

        rd_sb = cst.tile([P, tot_idx // 16], I16)
        nc.scalar.dma_start(rd_sb[:], rd_idx[:])

        # weights as lhsT chunks, pre-split into contiguous 128-wide m-tiles
        w_sb = cst.tile([P, D // P, CD // P, P], BF16)
        for k in range(D // P):
            for j in range(CD // P):
                nc.scalar.dma_start(
                    w_sb[:, k, j, :],
                    w_down[k * P:(k + 1) * P, j * P:(j + 1) * P])
        wc1_sb = cst.tile([P, CD // P, HD // P, P], BF16)
        for k in range(CD // P):
            for i in range(HD // P):
                nc.scalar.dma_start(
                    wc1_sb[:, k, i, :],
                    wc1[k * P:(k + 1) * P, i * P:(i + 1) * P])
        wc2_sb = cst.tile([P, HD // P, CD // P, P], BF16)
        for k in range(HD // P):
            for j in range(CD // P):
                nc.scalar.dma_start(
                    wc2_sb[:, k, j, :],
                    wc2[k * P:(k + 1) * P, j * P:(j + 1) * P])

        ones1 = cst.tile([1, WTILE], F32)
        nc.vector.memset(ones1[:], 1.0)
        bd_sb = cst.tile([1, CD], F32)
        nc.scalar.dma_start(bd_sb[:], b_down[:])
        bc1_sb = cst.tile([1, HD], F32)
        nc.scalar.dma_start(bc1_sb[:], bc1[:])
        bc2_sb = cst.tile([1, CD], F32)
        nc.scalar.dma_start(bc2_sb[:], bc2[:])

        # SBUF-resident transposed value log: two f32 planes (compute/gather)
        # plus a bf16 mirror used only for dumping to DRAM
        vT = [cst.tile([P, nslots], F32, name=f"vT{j}") for j in range(2)]
        vTb = cst.tile([P, nslots, 2], BF16)
        for j in range(2):
            nc.vector.memset(vT[j][:, 0:1], 0.0)
        nc.vector.memset(vTb[:, 0:1, :], 0.0)
        nc.scalar.dma_start(vlogT[:, 0:1, :], vTb[:, 0:1, :])

        def emit_transpose(ci):
            embT = sb.tile([P, D // P, ACHUNK], BF16, tag="embT", bufs=8)
            nc.sync.dma_start_transpose(
                embT[:], emb_s[ci * ACHUNK:(ci + 1) * ACHUNK, :])
            return embT

        def emit_a_compute(embT, s0, f32cast):
            for j in range(2):
                acc = ps.tile([P, ACHUNK], F32, tag="acc", bufs=3)
                if has_bd:
                    nc.tensor.matmul(acc[:], lhsT=bd_sb[:, j * P:(j + 1) * P],
                                     rhs=ones1[:, 0:ACHUNK],
                                     start=True, stop=False)
                for k in range(D // P):
                    nc.tensor.matmul(acc[:], lhsT=w_sb[:, k, j, :],
                                     rhs=embT[:, k, :],
                                     start=(k == 0 and not has_bd),
                                     stop=(k == D // P - 1))
                if f32cast:
                    nc.vector.tensor_copy(out=vT[j][:, s0:s0 + ACHUNK],
                                          in_=acc[:])
                nc.scalar.copy(out=vTb[:, s0:s0 + ACHUNK, j], in_=acc[:])

        def emit_tile(ti, idx_off):
            l, tbase, w = tiles[ti]
            bound = bounds[ti]
            idxs = rd_sb[:, idx_off:idx_off + 4 * w // 16]
            meanT = sb.tile([P, 2, w], BF16, tag=f"meanT{w}")
            for j in range(2):
                g = sb.tile([P, 4 * w], F32, tag=f"g{w}_{j}", bufs=2)
                nc.gpsimd.ap_gather(
                    g[:].unsqueeze(2), vT[j][:, 0:bound].unsqueeze(2),
                    idxs, channels=P, num_elems=bound, d=1, num_idxs=4 * w)
                s01 = sb.tile([P, w], F32, tag=f"s01_{w}_{j}")
                nc.vector.tensor_add(out=s01[:], in0=g[:, 0 * w:1 * w],
                                     in1=g[:, 1 * w:2 * w])
                s23 = sb.tile([P, w], F32, tag=f"s23_{w}_{j}")
                nc.vector.tensor_add(out=s23[:], in0=g[:, 2 * w:3 * w],
                                     in1=g[:, 3 * w:4 * w])
                nc.vector.tensor_add(out=meanT[:, j, :], in0=s01[:], in1=s23[:])

            hT = sb.tile([P, HD // P, w], BF16, tag=f"hT{w}", bufs=2)
            for i in range(HD // P):
                phb = ps.tile([P, WTILE], F32, tag="ph", bufs=2)
                ph = phb[:, 0:w]
                if has_b1:
                    nc.tensor.matmul(ph, lhsT=bc1_sb[:, i * P:(i + 1) * P],
                                     rhs=ones1[:, 0:w], start=True, stop=False)
                for k in range(CD // P):
                    nc.tensor.matmul(ph, lhsT=wc1_sb[:, k, i, :],
                                     rhs=meanT[:, k, :],
                                     start=(k == 0 and not has_b1),
                                     stop=(k == CD // P - 1))
                nc.scalar.activation(
                    out=hT[:, i, :], in_=ph,
                    func=mybir.ActivationFunctionType.Gelu_apprx_tanh,
                    scale=float(inv))
            for j in range(2):
                pob = ps.tile([P, WTILE], F32, tag="po", bufs=2)
                po = pob[:, 0:w]
                if has_b2:
                    nc.tensor.matmul(po, lhsT=bc2_sb[:, j * P:(j + 1) * P],
                                     rhs=ones1[:, 0:w], start=True, stop=False)
                for k in range(HD // P):
                    nc.tensor.matmul(po, lhsT=wc2_sb[:, k, j, :],
                                     rhs=hT[:, k, :],
                                     start=(k == 0 and not has_b2),
                                     stop=(k == HD // P - 1))
                if l < NLEV - 1:   # last level's outputs are never gathered
                    nc.vector.tensor_copy(out=vT[j][:, tbase:tbase + w],
                                          in_=po)
                nc.scalar.copy(out=vTb[:, tbase:tbase + w, j], in_=po)

        # dumps: each range emitted strictly AFTER its writers; A1 ranges
        # split over the sync/scalar queues, tile/A2 ranges alternate too
        def dump(eng, s0, w):
            eng.dma_start(vlogT[:, s0:s0 + w, :], vTb[:, s0:s0 + w, :])

        # ---- phase A1: compose-read token chunks ----
        embTs = {}
        for ci in range(A1_CHUNKS):
            embTs[ci] = emit_transpose(ci)
        for ci in range(A1_CHUNKS):
            emit_a_compute(embTs[ci], 1 + ci * ACHUNK, True)

        # A2 transposes go on sync right after A1's, then sync drains the
        # even A1 dumps (all their mirrors complete progressively)
        a2list = list(range(A1_CHUNKS, A_CHUNKS))
        for ci in a2list:
            embTs[ci] = emit_transpose(ci)
        for ci in range(0, A1_CHUNKS, 2):
            dump(nc.sync, 1 + ci * ACHUNK, ACHUNK)

        # ---- compose supertiles, A2 filler computes and the remaining dumps
        #      interleaved so no engine ever stalls the pipeline ----
        odd_a1 = [1 + ci * ACHUNK for ci in range(1, A1_CHUNKS, 2)]
        nsd = max(1, -(-len(odd_a1) // len(tiles)))
        idx_off = 0
        for ti in range(len(tiles)):
            emit_tile(ti, idx_off)
            _, tbase, w = tiles[ti]
            idx_off += 4 * w // 16
            dump(nc.sync if ti % 2 == 0 else nc.scalar, tbase, w)
            if ti < len(a2list):
                ci = a2list[ti]
                s0 = fill_base + (ci - A1_CHUNKS) * ACHUNK
                emit_a_compute(embTs[ci], s0, False)
                dump(nc.sync if ti % 2 == 0 else nc.scalar, s0, ACHUNK)
            for s0 in odd_a1[ti * nsd:(ti + 1) * nsd]:
                dump(nc.scalar, s0, ACHUNK)
        for k, ci in enumerate(a2list[len(tiles):]):
            s0 = fill_base + (ci - A1_CHUNKS) * ACHUNK
            emit_a_compute(embTs[ci], s0, False)
            dump(nc.sync if k % 2 == 0 else nc.scalar, s0, ACHUNK)
        for s0 in odd_a1[len(tiles) * nsd:]:
            dump(nc.scalar, s0, ACHUNK)

    nc.compile()
    return nc


_CACHE = {}


def _get_bass(key):
    if key not in _CACHE:
        _CACHE[key] = build_bass(*key)
    return _CACHE[key]


def _install_ntff_hook():
    try:
        import antenv.axon_hooks  # noqa: F401
        return
    except ImportError:
        pass
    try:
        import trn_agent_boot.trn_boot as _tb
        hooks = types.ModuleType('antenv.axon_hooks')
        hook = _tb._ntff_profile_via_ctypes('/opt/axon/libaxon_pjrt.so')
        hooks.get_axon_ntff_profile_hook = lambda: hook
        hooks.set_axon_ntff_profile_hook = lambda h: None
        sys.modules['antenv.axon_hooks'] = hooks
    except Exception:
        pass


def run(inputs, trace=False):
    """Returns (full_output, exec_time_ns or None)."""
    inp = {k: (np.asarray(v) if hasattr(v, 'shape') else v)
           for k, v in inputs.items()}
    spans_list = [inp["spans0"], inp["spans1"], inp["spans2"]]
    cores, meta = plan(inp["chunk_input_ids"], spans_list)

    def f32(x):
        return np.ascontiguousarray(x, np.float32)

    b_down = f32(inp["b_down"]).reshape(1, CD)
    bc1 = f32(inp["bc1"]).reshape(1, HD)
    bc2 = f32(inp["bc2"]).reshape(1, CD)
    has_bd = bool(np.any(b_down))
    has_b1 = bool(np.any(bc1))
    has_b2 = bool(np.any(bc2))

    nc = _get_bass((meta["NTOKP"], meta["NCTOKP"], meta["A1_CHUNKS"],
                    meta["A_CHUNKS"], meta["fill_base"], meta["tiles"],
                    meta["bounds"], meta["nslots"],
                    has_bd, has_b1, has_b2, meta["inv"]))

    emb_bf = np.asarray(inp["emb_table"], np.float32).astype(ml_dtypes.bfloat16)

    def bf16(x):
        return np.ascontiguousarray(
            np.asarray(x, np.float32).astype(ml_dtypes.bfloat16))

    shared = dict(
        w_down=bf16(inp["w_down"]),
        b_down=b_down,
        wc1=bf16(inp["wc1"]),
        bc1=bc1,
        wc2=bf16(inp["wc2"]),
        bc2=bc2,
    )
    NTOKP, NCTOKP = meta["NTOKP"], meta["NCTOKP"]
    in_maps = []
    for c in range(N_CORES):
        core = cores[c]
        m = dict(shared)
        stream = np.zeros((NTOKP, D), ml_dtypes.bfloat16)
        ct, fl = core["ctok"], core["fill"]
        if len(ct):
            stream[:len(ct)] = emb_bf[np.asarray(ct, np.int64)]
        if len(fl):
            stream[NCTOKP:NCTOKP + len(fl)] = emb_bf[np.asarray(fl, np.int64)]
        m["emb_s"] = stream
        m["rd_idx"] = wrap_idx16(core["rd"])
        in_maps.append(m)

    _install_ntff_hook()
    res = run_bass_kernel_spmd(nc, in_maps, core_ids=list(range(N_CORES)),
                               trace=trace)
    vals = []
    for c in range(N_CORES):
        d = np.asarray(res.results[c]["vlogT"]).astype(np.float32)
        vals.append(d.transpose(1, 2, 0).reshape(-1, CD))   # [nslots, 256]
    vals = np.stack(vals)                           # [8, nslots, 256]
    full = vals[meta["pos_core"], meta["pos_slot"]]
    return full.reshape(16, 2048, CD), res.exec_time_ns


def kernel(**inputs):
    out, _ = run(inputs, trace=False)
    return out


# revision 11
# speedup vs baseline: 1.1970x; 1.1862x over previous
"""Trainium2 Bass kernel for the n-ary span-compose problem (gnn_message_passing).

Strategy v3 (zero cross-core communication, host-planned, no dma_gather):
  The host resolves the full version DAG (which value every compose reads and
  which write wins each output position).  Needed composes form tiny connected
  components, distributed over 8 cores balancing MLP work and embedding-stream
  length (with token-overlap-aware clustering to cut duplication).

  Each core keeps a TRANSPOSED value log resident in SBUF as two f32 planes:
      vT[j][p, s] = value_of_slot_s[j*128 + p]   (2 x [128, nslots] f32)
  slot space: 0 = zeros (pad reads), [1, 1+NCTOKP) = compose-read tokens,
  then compose outputs level by level, then base-final filler tokens (so
  compose gather bounds never cover fillers).

  Phase A: the per-core token stream is compacted ON HOST into a dense
  [NTOKP, 768] bf16 input and streamed with xbar transpose DMA
  (dma_start_transpose -> pre-transposed lhsT-ready tiles, no GpSimd
  descriptor generation).  Down-projection runs as a transposed GEMM
  (lhsT = w_down) writing straight into vT0/vT1.

  Compose tiles (width 256/128) fetch their 4 operands per compose per plane
  with ap_gather (GpSimd SIMD ucode gather from SBUF along the free dim),
  sum with 3 contiguous DVE adds per plane (last add writes the bf16
  transposed mean), fold the 1/cnt mean scale into the GELU's scale
  argument, and run both MLP layers as transposed GEMMs (lhsT = wc1 / wc2)
  so no PE transposes are needed anywhere.  Outputs are copied from PSUM
  straight into vT0/vT1.

  The logs are dumped to DRAM incrementally on the scalar engine (the sync
  engine does nothing but the xbar stream); the host assembles the final
  [16, 2048, 256] output from (core, slot) maps.  Filler A-chunks are
  emitted after the compose tiles so the PE stream has no gaps.
"""

import sys
import types
import numpy as np
import ml_dtypes
from contextlib import ExitStack

import concourse.bass as bass
import concourse.bacc as bacc
import concourse.mybir as mybir
import concourse.tile as tile
from concourse.bass_utils import run_bass_kernel_spmd

N_CORES = 8
NPOS = 16 * 2048
NLEV = 3
NSPAN = 4096
VOCAB = 32000
D = 768
CD = 256
HD = 1024
P = 128
F32 = mybir.dt.float32
BF16 = mybir.dt.bfloat16
I16 = mybir.dt.int16

ACHUNK = 256      # rows per phase-A stream chunk
WTILE = 256       # composes per supertile (last tile of a level may be 128)


# --------------------------------------------------------------------------
# host planner
# --------------------------------------------------------------------------

def _last_wins(tgt):
    u, first_rev = np.unique(tgt[::-1], return_index=True)
    return u, len(tgt) - 1 - first_rev


def plan(chunk_input_ids, spans_list):
    ids = np.asarray(chunk_input_ids).astype(np.int64).ravel()
    ids = np.where(ids == -100, 0, ids)
    assert ids.size == NPOS

    # ---- version DAG ----
    ver = np.arange(NPOS, dtype=np.int64)
    comp_reads, comp_cnt = [], []
    for l, spans in enumerate(spans_list):
        spans = np.asarray(spans).astype(np.int64)
        mask = spans != -100
        tgt = spans.max(-1) + 1
        idx = np.where(mask, spans, 0)
        rd = np.where(mask, ver[idx], -1)
        comp_reads.append(rd)
        comp_cnt.append(mask.sum(-1))
        u, win = _last_wins(tgt)
        ver[u] = NPOS + l * NSPAN + win
    final_ver = ver

    # ---- liveness ----
    needed = [np.zeros(NSPAN, bool) for _ in range(NLEV)]
    fin_comp = final_ver[final_ver >= NPOS] - NPOS
    for l in range(NLEV):
        needed[l][fin_comp[fin_comp // NSPAN == l] % NSPAN] = True
    for l in range(NLEV - 1, -1, -1):
        rd = comp_reads[l][needed[l]].ravel()
        rd = rd[rd >= NPOS] - NPOS
        for l2 in range(l):
            needed[l2][rd[rd // NSPAN == l2] % NSPAN] = True

    # ---- connected components over comp->comp read edges ----
    parent = {}

    def find(x):
        root = x
        while parent[root] != root:
            root = parent[root]
        while parent[x] != root:
            parent[x], x = root, parent[x]
        return root

    for l in range(NLEV):
        for r in np.nonzero(needed[l])[0]:
            parent[l * NSPAN + r] = l * NSPAN + r
    for l in range(NLEV):
        rows = np.nonzero(needed[l])[0]
        rd = comp_reads[l][rows]
        for i, r in enumerate(rows):
            for v in rd[i]:
                if v >= NPOS:
                    ra, rb = find(l * NSPAN + int(r)), find(int(v - NPOS))
                    if ra != rb:
                        parent[ra] = rb

    comps_by_root = {}
    for node in parent:
        comps_by_root.setdefault(find(node), []).append(node)

    # ---- group metadata: per-level comp counts + compose-read token sets ----
    groups = []
    for g in comps_by_root.values():
        per_lvl = np.zeros(NLEV, np.int64)
        toks = set()
        for uid in g:
            l = uid // NSPAN
            per_lvl[l] += 1
            for v in comp_reads[l][uid % NSPAN]:
                v = int(v)
                if 0 <= v < NPOS:
                    toks.add(int(ids[v]))
        groups.append((g, per_lvl, toks))

    # ---- greedy assignment: balance MLP comps + token stream, cluster by
    #      token overlap (newtok term) ----
    WC, WT = 18.5, 7.0   # ~ns per compose (MLP) / per streamed token row
    comp_core = {}
    compload = np.zeros((N_CORES, NLEV))
    tokload = np.zeros(N_CORES)
    tok_sets = [set() for _ in range(N_CORES)]
    order = sorted(range(len(groups)),
                   key=lambda i: -(len(groups[i][0]) * 4 + len(groups[i][2])))
    for gi in order:
        g, per_lvl, toks = groups[gi]
        best, bestc = None, 0
        for c in range(N_CORES):
            newtok = sum(1 for t in toks if t not in tok_sets[c])
            score = (WC * (compload[c].sum() + per_lvl.sum())
                     + WT * (tokload[c] + newtok)
                     + 0.25 * WC * (compload[c] + per_lvl).max())
            if best is None or score < best:
                best, bestc = score, c
        c = bestc
        for uid in g:
            comp_core[uid] = c
        compload[c] += per_lvl
        tokload[c] += sum(1 for t in toks if t not in tok_sets[c])
        tok_sets[c].update(toks)

    # ---- base-final tokens: canonical core (prefer one that has it) ----
    is_comp_final = final_ver >= NPOS
    base_pos = np.nonzero(~is_comp_final)[0]
    tok_canon = {}
    filler = [[] for _ in range(N_CORES)]
    fill_load = np.zeros(N_CORES, np.int64)
    for p in base_pos:
        t = int(ids[p])
        if t in tok_canon:
            continue
        for c in range(N_CORES):
            if t in tok_sets[c]:
                tok_canon[t] = c
                break
        else:
            c = int(np.argmin(fill_load))
            tok_canon[t] = c
            filler[c].append(t)
            fill_load[c] += 1

    # ---- per-core streams / slots / tiles ----
    def rup(x, m):
        return -(-int(x) // m) * m

    core_ctok = []     # compose-read tokens in first-use order
    for c in range(N_CORES):
        lst, seen = [], set()
        for l in range(NLEV):
            rows = sorted(uid % NSPAN for uid, cc in comp_core.items()
                          if cc == c and uid // NSPAN == l)
            for r in rows:
                for v in comp_reads[l][r]:
                    v = int(v)
                    if 0 <= v < NPOS:
                        t = int(ids[v])
                        if t not in seen:
                            seen.add(t)
                            lst.append(t)
        core_ctok.append(lst)

    NCTOKP = rup(max(len(l) for l in core_ctok), ACHUNK)
    FILLP = rup(max(len(f) for f in filler), ACHUNK)
    A1_CHUNKS = NCTOKP // ACHUNK
    A_CHUNKS = A1_CHUNKS + FILLP // ACHUNK
    NTOKP = A_CHUNKS * ACHUNK

    ncmp = np.zeros((N_CORES, NLEV), np.int64)
    for uid, c in comp_core.items():
        ncmp[c, uid // NSPAN] += 1
    NC = [int(rup(ncmp[:, l].max(), P)) for l in range(NLEV)]
    lvl_base = []
    b = 1 + NCTOKP
    for l in range(NLEV):
        lvl_base.append(b)
        b += NC[l]
    fill_base = b
    nslots = b + FILLP
    assert nslots < 32768

    # tile widths per level (shared across cores)
    tiles = []   # list of (level, base_slot, W)
    for l in range(NLEV):
        off = 0
        while off < NC[l]:
            w = WTILE if NC[l] - off >= WTILE else P
            tiles.append((l, lvl_base[l] + off, w))
            off += w

    inv_vals = set()
    core_rd = []
    core_bounds = []
    core_slot_of_comp = []
    core_tok_slot = []
    for c in range(N_CORES):
        slot_of_tok = {t: 1 + i for i, t in enumerate(core_ctok[c])}
        for i, t in enumerate(filler[c]):
            slot_of_tok[t] = fill_base + i
        core_tok_slot.append(slot_of_tok)
        slot_of_comp = {}
        rd_all = []
        bounds = []

        def vslot(v):
            v = int(v)
            if v == -1:
                return 0
            if v < NPOS:
                return slot_of_tok[int(ids[v])]
            return slot_of_comp[v - NPOS]

        for l in range(NLEV):
            rows = sorted(uid % NSPAN for uid, cc in comp_core.items()
                          if cc == c and uid // NSPAN == l)

            def row_bound(r):
                return max((vslot(v) for v in comp_reads[l][r]), default=0)
            rows = sorted(rows, key=lambda r: (row_bound(r), r))
            for i, r in enumerate(rows):
                slot_of_comp[l * NSPAN + int(r)] = lvl_base[l] + i
                inv_vals.add(1.0 / max(int(comp_cnt[l][r]), 1))
            rs = np.zeros((NC[l], 4), np.int64)
            for i, r in enumerate(rows):
                for k in range(4):
                    rs[i, k] = vslot(comp_reads[l][r, k])
            off = 0
            for (tl, tbase, w) in tiles:
                if tl != l:
                    continue
                blk = rs[off:off + w]          # [w, 4]
                rd_all.append(blk.T.reshape(-1))   # k-major [4*w]
                bounds.append(max(1, int(blk.max()) + 1))
                off += w
        core_rd.append(np.concatenate(rd_all))
        core_bounds.append(bounds)
        core_slot_of_comp.append(slot_of_comp)

    bounds = tuple(max(core_bounds[c][i] for c in range(N_CORES))
                   for i in range(len(tiles)))
    for i, (_, tbase, w) in enumerate(tiles):
        assert bounds[i] <= tbase

    if not inv_vals:
        inv_vals = {0.25}
    assert len(inv_vals) == 1, f"non-uniform span counts {inv_vals}"
    inv_uniform = float(inv_vals.pop())

    # ---- output assembly maps ----
    pos_core = np.empty(NPOS, np.int64)
    pos_slot = np.empty(NPOS, np.int64)
    for p in range(NPOS):
        v = int(final_ver[p])
        if v < NPOS:
            t = int(ids[v])
            c = tok_canon[t]
            pos_core[p] = c
            pos_slot[p] = core_tok_slot[c][t]
        else:
            c = comp_core[v - NPOS]
            pos_core[p] = c
            pos_slot[p] = core_slot_of_comp[c][v - NPOS]

    cores = []
    for c in range(N_CORES):
        cores.append(dict(ctok=core_ctok[c], fill=filler[c], rd=core_rd[c]))
    meta = dict(NTOKP=NTOKP, NCTOKP=NCTOKP, A1_CHUNKS=A1_CHUNKS,
                A_CHUNKS=A_CHUNKS, fill_base=fill_base,
                tiles=tuple(tiles), bounds=bounds, nslots=nslots,
                inv=inv_uniform, pos_core=pos_core, pos_slot=pos_slot)
    return cores, meta


def wrap_idx16(idx):
    """[n] -> [128, n/16] int16 layout for gpsimd gathers (i -> (i%16, i//16))."""
    idx = np.asarray(idx, np.int64)
    n = len(idx)
    assert n % 16 == 0 and idx.max() < 32768 and idx.min() >= 0
    w = idx.reshape(n // 16, 16).T.astype(np.int16)
    return np.tile(w, (8, 1))


# --------------------------------------------------------------------------
# bass program
# --------------------------------------------------------------------------

def build_bass(NTOKP, NCTOKP, A1_CHUNKS, A_CHUNKS, fill_base, tiles, bounds,
               nslots, has_bd, has_b1, has_b2, inv):
    nc = bacc.Bacc("TRN2", target_bir_lowering=False, debug=False,
                   num_devices=N_CORES, num_swdge_queues=4)

    QCH = 4 * ACHUNK   # stream-load quarter size (columns)
    emb_sT = nc.dram_tensor("emb_sT", [D // P, P, NTOKP], BF16,
                            kind="ExternalInput")
    w_dT = nc.dram_tensor("w_dT", [P, D // P, CD // P, P], BF16,
                          kind="ExternalInput")
    b_down = nc.dram_tensor("b_down", [1, CD], F32, kind="ExternalInput")
    wc1T = nc.dram_tensor("wc1T", [P, CD // P, HD // P, P], BF16,
                          kind="ExternalInput")
    bc1 = nc.dram_tensor("bc1", [1, HD], F32, kind="ExternalInput")
    wc2T = nc.dram_tensor("wc2T", [P, HD // P, CD // P, P], BF16,
                          kind="ExternalInput")
    bc2 = nc.dram_tensor("bc2", [1, CD], F32, kind="ExternalInput")
    tot_idx = sum(4 * w for (_, _, w) in tiles)
    rd_idx = nc.dram_tensor("rd_idx", [P, tot_idx // 16], I16,
                            kind="ExternalInput")
    vlogT0 = nc.dram_tensor("vlogT0", [P, nslots], F32, kind="ExternalOutput")
    vlogT1 = nc.dram_tensor("vlogT1", [P, nslots], F32, kind="ExternalOutput")
    vlogT = [vlogT0, vlogT1]

    with tile.TileContext(nc) as tc, ExitStack() as ctx:
        cst = ctx.enter_context(tc.tile_pool(name="cst", bufs=1))
        sb = ctx.enter_context(tc.tile_pool(name="sb", bufs=3))
        ps = ctx.enter_context(tc.tile_pool(name="ps", bufs=2, space="PSUM"))

        rd_sb = cst.tile([P, tot_idx // 16], I16)
        nc.scalar.dma_start(rd_sb[:], rd_idx[:])

        # weights: host-prearranged lhsT layouts, one DMA each
        w_sb = cst.tile([P, D // P, CD // P, P], BF16)
        nc.scalar.dma_start(w_sb[:], w_dT[:])
        wc1_sb = cst.tile([P, CD // P, HD // P, P], BF16)
        nc.scalar.dma_start(wc1_sb[:], wc1T[:])
        wc2_sb = cst.tile([P, HD // P, CD // P, P], BF16)
        nc.scalar.dma_start(wc2_sb[:], wc2T[:])

        ones1 = cst.tile([1, WTILE], F32)
        nc.vector.memset(ones1[:], 1.0)
        bd_sb = cst.tile([1, CD], F32)
        nc.scalar.dma_start(bd_sb[:], b_down[:])
        bc1_sb = cst.tile([1, HD], F32)
        nc.scalar.dma_start(bc1_sb[:], bc1[:])
        bc2_sb = cst.tile([1, CD], F32)
        nc.scalar.dma_start(bc2_sb[:], bc2[:])

        # the whole pre-transposed embedding stream, SBUF-resident
        embT = cst.tile([P, D // P, NTOKP], BF16)
        nq = NTOKP // QCH
        for q in range(nq):
            eng = nc.sync if q % 2 == 0 else nc.scalar
            for k in range(D // P):
                eng.dma_start(embT[:, k, q * QCH:(q + 1) * QCH],
                              emb_sT[k, :, q * QCH:(q + 1) * QCH])

        # SBUF-resident transposed value log: two f32 planes
        vT = [cst.tile([P, nslots], F32, name=f"vT{j}") for j in range(2)]
        for j in range(2):
            nc.vector.memset(vT[j][:, 0:1], 0.0)
            nc.scalar.dma_start(vlogT[j][:, 0:1], vT[j][:, 0:1])

        def emit_a_compute(ci, s0):
            for j in range(2):
                acc = ps.tile([P, ACHUNK], F32, tag="acc", bufs=3)
                if has_bd:
                    nc.tensor.matmul(acc[:], lhsT=bd_sb[:, j * P:(j + 1) * P],
                                     rhs=ones1[:, 0:ACHUNK],
                                     start=True, stop=False)
                for k in range(D // P):
                    nc.tensor.matmul(
                        acc[:], lhsT=w_sb[:, k, j, :],
                        rhs=embT[:, k, ci * ACHUNK:(ci + 1) * ACHUNK],
                        start=(k == 0 and not has_bd),
                        stop=(k == D // P - 1))
                nc.vector.tensor_copy(out=vT[j][:, s0:s0 + ACHUNK], in_=acc[:])

        def emit_tile(ti, idx_off):
            l, tbase, w = tiles[ti]
            bound = bounds[ti]
            idxs = rd_sb[:, idx_off:idx_off + 4 * w // 16]
            meanT = sb.tile([P, 2, w], BF16, tag=f"meanT{w}")
            for j in range(2):
                g = sb.tile([P, 4 * w], F32, tag=f"g{w}_{j}", bufs=2)
                nc.gpsimd.ap_gather(
                    g[:].unsqueeze(2), vT[j][:, 0:bound].unsqueeze(2),
                    idxs, channels=P, num_elems=bound, d=1, num_idxs=4 * w)
                s01 = sb.tile([P, w], F32, tag=f"s01_{w}_{j}")
                nc.vector.tensor_add(out=s01[:], in0=g[:, 0 * w:1 * w],
                                     in1=g[:, 1 * w:2 * w])
                s23 = sb.tile([P, w], F32, tag=f"s23_{w}_{j}")
                nc.vector.tensor_add(out=s23[:], in0=g[:, 2 * w:3 * w],
                                     in1=g[:, 3 * w:4 * w])
                nc.vector.tensor_add(out=meanT[:, j, :], in0=s01[:], in1=s23[:])

            hT = sb.tile([P, HD // P, w], BF16, tag=f"hT{w}", bufs=2)
            for i2 in range(0, HD // P, 2):
                phb = ps.tile([P, 2 * WTILE], F32, tag="ph", bufs=2)
                for di in range(2):
                    ph = phb[:, di * w:(di + 1) * w]
                    i = i2 + di
                    if has_b1:
                        nc.tensor.matmul(ph, lhsT=bc1_sb[:, i * P:(i + 1) * P],
                                         rhs=ones1[:, 0:w],
                                         start=True, stop=False)
                    for k in range(CD // P):
                        nc.tensor.matmul(ph, lhsT=wc1_sb[:, k, i, :],
                                         rhs=meanT[:, k, :],
                                         start=(k == 0 and not has_b1),
                                         stop=(k == CD // P - 1))
                nc.scalar.activation(
                    out=hT[:, i2:i2 + 2, :], in_=phb[:, 0:2 * w],
                    func=mybir.ActivationFunctionType.Gelu_apprx_tanh,
                    scale=float(inv))
            for j in range(2):
                pob = ps.tile([P, WTILE], F32, tag="po", bufs=2)
                po = pob[:, 0:w]
                if has_b2:
                    nc.tensor.matmul(po, lhsT=bc2_sb[:, j * P:(j + 1) * P],
                                     rhs=ones1[:, 0:w], start=True, stop=False)
                for k in range(HD // P):
                    nc.tensor.matmul(po, lhsT=wc2_sb[:, k, j, :],
                                     rhs=hT[:, k, :],
                                     start=(k == 0 and not has_b2),
                                     stop=(k == HD // P - 1))
                nc.vector.tensor_copy(out=vT[j][:, tbase:tbase + w], in_=po)

        def dump(s0, w):
            nc.sync.dma_start(vlogT[0][:, s0:s0 + w], vT[0][:, s0:s0 + w])
            nc.scalar.dma_start(vlogT[1][:, s0:s0 + w], vT[1][:, s0:s0 + w])

        # ---- phase A1: compose-read token chunks ----
        for ci in range(A1_CHUNKS):
            emit_a_compute(ci, 1 + ci * ACHUNK)
            if ci % 4 == 3 or ci == A1_CHUNKS - 1:
                g0 = (ci // 4) * 4
                dump(1 + g0 * ACHUNK, (ci - g0 + 1) * ACHUNK)

        # ---- compose supertiles with A2 filler chunks interleaved ----
        a2list = list(range(A1_CHUNKS, A_CHUNKS))
        idx_off = 0
        for ti in range(len(tiles)):
            emit_tile(ti, idx_off)
            _, tbase, w = tiles[ti]
            idx_off += 4 * w // 16
            dump(tbase, w)
            if ti < len(a2list):
                ci = a2list[ti]
                emit_a_compute(ci, fill_base + (ci - A1_CHUNKS) * ACHUNK)
        for ci in a2list[len(tiles):]:
            emit_a_compute(ci, fill_base + (ci - A1_CHUNKS) * ACHUNK)
        if a2list:
            dump(fill_base, len(a2list) * ACHUNK)

    nc.compile()
    return nc


_CACHE = {}


def _get_bass(key):
    if key not in _CACHE:
        _CACHE[key] = build_bass(*key)
    return _CACHE[key]


def _install_ntff_hook():
    try:
        import antenv.axon_hooks  # noqa: F401
        return
    except ImportError:
        pass
    try:
        import trn_agent_boot.trn_boot as _tb
        hooks = types.ModuleType('antenv.axon_hooks')
        hook = _tb._ntff_profile_via_ctypes('/opt/axon/libaxon_pjrt.so')
        hooks.get_axon_ntff_profile_hook = lambda: hook
        hooks.set_axon_ntff_profile_hook = lambda h: None
        sys.modules['antenv.axon_hooks'] = hooks
    except Exception:
        pass


def run(inputs, trace=False):
    """Returns (full_output, exec_time_ns or None)."""
    inp = {k: (np.asarray(v) if hasattr(v, 'shape') else v)
           for k, v in inputs.items()}
    spans_list = [inp["spans0"], inp["spans1"], inp["spans2"]]
    cores, meta = plan(inp["chunk_input_ids"], spans_list)

    def f32(x):
        return np.ascontiguousarray(x, np.float32)

    b_down = f32(inp["b_down"]).reshape(1, CD)
    bc1 = f32(inp["bc1"]).reshape(1, HD)
    bc2 = f32(inp["bc2"]).reshape(1, CD)
    has_bd = bool(np.any(b_down))
    has_b1 = bool(np.any(bc1))
    has_b2 = bool(np.any(bc2))

    nc = _get_bass((meta["NTOKP"], meta["NCTOKP"], meta["A1_CHUNKS"],
                    meta["A_CHUNKS"], meta["fill_base"], meta["tiles"],
                    meta["bounds"], meta["nslots"],
                    has_bd, has_b1, has_b2, meta["inv"]))

    emb_bf = np.asarray(inp["emb_table"], np.float32).astype(ml_dtypes.bfloat16)

    def bf16(x):
        return np.ascontiguousarray(
            np.asarray(x, np.float32).astype(ml_dtypes.bfloat16))

    w_dT = bf16(inp["w_down"]).reshape(6, P, 2, P).transpose(1, 0, 2, 3)
    wc1T = bf16(inp["wc1"]).reshape(2, P, 8, P).transpose(1, 0, 2, 3)
    wc2T = bf16(inp["wc2"]).reshape(8, P, 2, P).transpose(1, 0, 2, 3)
    shared = dict(
        w_dT=np.ascontiguousarray(w_dT),
        b_down=b_down,
        wc1T=np.ascontiguousarray(wc1T),
        bc1=bc1,
        wc2T=np.ascontiguousarray(wc2T),
        bc2=bc2,
    )
    NTOKP, NCTOKP = meta["NTOKP"], meta["NCTOKP"]
    in_maps = []
    for c in range(N_CORES):
        core = cores[c]
        m = dict(shared)
        stream = np.zeros((NTOKP, D), ml_dtypes.bfloat16)
        ct, fl = core["ctok"], core["fill"]
        if len(ct):
            stream[:len(ct)] = emb_bf[np.asarray(ct, np.int64)]
        if len(fl):
            stream[NCTOKP:NCTOKP + len(fl)] = emb_bf[np.asarray(fl, np.int64)]
        m["emb_sT"] = np.ascontiguousarray(
            stream.reshape(NTOKP, 6, P).transpose(1, 2, 0))
        m["rd_idx"] = wrap_idx16(core["rd"])
        in_maps.append(m)

    _install_ntff_hook()
    res = run_bass_kernel_spmd(nc, in_maps, core_ids=list(range(N_CORES)),
                               trace=trace)
    vals = []
    for c in range(N_CORES):
        d0 = np.asarray(res.results[c]["vlogT0"])   # [128, nslots]
        d1 = np.asarray(res.results[c]["vlogT1"])
        vals.append(np.hstack([d0.T, d1.T]))        # [nslots, 256]
    vals = np.stack(vals)                           # [8, nslots, 256]
    full = vals[meta["pos_core"], meta["pos_slot"]]
    return full.reshape(16, 2048, CD), res.exec_time_ns


def kernel(**inputs):
    out, _ = run(inputs, trace=False)
    return out


# revision 13
# speedup vs baseline: 3.3866x; 2.8292x over previous
"""Trainium2 Bass kernel for the n-ary span-compose problem (gnn_message_passing).

Strategy v8 (zero cross-core communication, host-planned):
  The host resolves the full version DAG (which value every compose reads and
  which write wins each output position).  Needed composes form tiny connected
  components, distributed over 8 cores balancing MLP work and embedding-stream
  length (token-overlap-aware clustering cuts duplication).

  Per core, the host builds a PRE-TRANSPOSED embedding stream (bf16,
  [6, 128, NSTREAM]): level-0 operand instances laid out per-tile k-major,
  followed by the deduplicated tokens read by level-1/2 composes and the
  base-final canonical tokens.  The device loads it with a few big plain
  DMAs into a resident SBUF tile (no descriptor-generation bottlenecks, no
  xbar, no gathers for phase A).

  Values live in a row-major DRAM log  vlog[slot, 256] (bf16) that doubles
  as the kernel output:
    slot 0 = zeros, [1, 1+NDTOKP) = deduped tokens, then L0/L1/L2 composes.
  - Deduped tokens: normal GEMM (lhsT = stream slices, rhs = w_down),
    batched log writes.
  - L0 composes: the 4-operand mean is FUSED into the down-projection -- the
    four k-sections of the per-instance stream accumulate into one PSUM tile,
    yielding the transposed mean directly (no gather, no adds).
  - L1/L2 composes: operands fetched with dma_gather(transpose=True) from
    vlog (SWDGE descriptor gen ~9ns/idx, proven fast), 2 gathers per tile
    (k-pairs), 3 contiguous DVE adds -> transposed mean.
  - MLP: layer 1 transposed (lhsT = wc1 -> hT), gelu on PSUM pairs with the
    1/cnt mean scale folded into the activation's scale argument, layer 2
    normal (lhsT = hT chunks, rhs = wc2) -> row-major outputs written
    straight back to the log.
  The host assembles the final [16, 2048, 256] output from (core, slot).
"""

import sys
import types
import numpy as np
import ml_dtypes
from contextlib import ExitStack

import concourse.bass as bass
import concourse.bacc as bacc
import concourse.mybir as mybir
import concourse.tile as tile
from concourse.bass_utils import run_bass_kernel_spmd

N_CORES = 8
NPOS = 16 * 2048
NLEV = 3
NSPAN = 4096
VOCAB = 32000
D = 768
CD = 256
HD = 1024
P = 128
F32 = mybir.dt.float32
BF16 = mybir.dt.bfloat16
I16 = mybir.dt.int16

ABATCH = 512      # token slots per A-phase log-write batch
WTILE = 256       # composes per supertile (last tile of a level may be 128)


def _last_wins(tgt):
    u, first_rev = np.unique(tgt[::-1], return_index=True)
    return u, len(tgt) - 1 - first_rev


def _rup(x, m):
    return -(-int(x) // m) * m


# --------------------------------------------------------------------------
# host planner
# --------------------------------------------------------------------------

def plan(chunk_input_ids, spans_list):
    ids = np.asarray(chunk_input_ids).astype(np.int64).ravel()
    ids = np.where(ids == -100, 0, ids)
    assert ids.size == NPOS

    # ---- version DAG ----
    ver = np.arange(NPOS, dtype=np.int64)
    comp_reads, comp_cnt = [], []
    for l, spans in enumerate(spans_list):
        spans = np.asarray(spans).astype(np.int64)
        mask = spans != -100
        tgt = spans.max(-1) + 1
        idx = np.where(mask, spans, 0)
        rd = np.where(mask, ver[idx], -1)
        comp_reads.append(rd)
        comp_cnt.append(mask.sum(-1))
        u, win = _last_wins(tgt)
        ver[u] = NPOS + l * NSPAN + win
    final_ver = ver

    # ---- liveness ----
    needed = [np.zeros(NSPAN, bool) for _ in range(NLEV)]
    fin_comp = final_ver[final_ver >= NPOS] - NPOS
    for l in range(NLEV):
        needed[l][fin_comp[fin_comp // NSPAN == l] % NSPAN] = True
    for l in range(NLEV - 1, -1, -1):
        rd = comp_reads[l][needed[l]].ravel()
        rd = rd[rd >= NPOS] - NPOS
        for l2 in range(l):
            needed[l2][rd[rd // NSPAN == l2] % NSPAN] = True

    # ---- connected components over comp->comp read edges ----
    parent = {}

    def find(x):
        root = x
        while parent[root] != root:
            root = parent[root]
        while parent[x] != root:
            parent[x], x = root, parent[x]
        return root

    for l in range(NLEV):
        for r in np.nonzero(needed[l])[0]:
            parent[l * NSPAN + r] = l * NSPAN + r
    for l in range(NLEV):
        rows = np.nonzero(needed[l])[0]
        rd = comp_reads[l][rows]
        for i, r in enumerate(rows):
            for v in rd[i]:
                if v >= NPOS:
                    ra, rb = find(l * NSPAN + int(r)), find(int(v - NPOS))
                    if ra != rb:
                        parent[ra] = rb

    comps_by_root = {}
    for node in parent:
        comps_by_root.setdefault(find(node), []).append(node)

    # ---- group metadata ----
    groups = []
    for g in comps_by_root.values():
        per_lvl = np.zeros(NLEV, np.int64)
        toks = set()      # only L1/L2-read tokens matter for dedup load
        n_l0 = 0
        for uid in g:
            l = uid // NSPAN
            per_lvl[l] += 1
            for v in comp_reads[l][uid % NSPAN]:
                v = int(v)
                if 0 <= v < NPOS:
                    if l == 0:
                        n_l0 += 1
                    else:
                        toks.add(int(ids[v]))
        groups.append((g, per_lvl, toks, n_l0))

    # ---- greedy assignment ----
    WC, WT = 18.5, 7.0
    comp_core = {}
    compload = np.zeros((N_CORES, NLEV))
    tokload = np.zeros(N_CORES)
    tok_sets = [set() for _ in range(N_CORES)]
    order = sorted(range(len(groups)),
                   key=lambda i: -(len(groups[i][0]) * 4 + len(groups[i][2])))
    for gi in order:
        g, per_lvl, toks, n_l0 = groups[gi]
        best, bestc = None, 0
        for c in range(N_CORES):
            newtok = sum(1 for t in toks if t not in tok_sets[c])
            score = (WC * (compload[c].sum() + per_lvl.sum())
                     + WT * (tokload[c] + newtok + n_l0)
                     + 0.25 * WC * (compload[c] + per_lvl).max())
            if best is None or score < best:
                best, bestc = score, c
        c = bestc
        for uid in g:
            comp_core[uid] = c
        compload[c] += per_lvl
        tokload[c] += sum(1 for t in toks if t not in tok_sets[c]) + n_l0
        tok_sets[c].update(toks)

    # ---- base-final canonical tokens ----
    is_comp_final = final_ver >= NPOS
    base_pos = np.nonzero(~is_comp_final)[0]
    tok_canon = {}
    extra = [[] for _ in range(N_CORES)]
    ex_load = np.array([len(s) for s in tok_sets], np.int64)
    for p in base_pos:
        t = int(ids[p])
        if t in tok_canon:
            continue
        for c in range(N_CORES):
            if t in tok_sets[c]:
                tok_canon[t] = c
                break
        else:
            c = int(np.argmin(ex_load))
            tok_canon[t] = c
            extra[c].append(t)
            ex_load[c] += 1

    # ---- shared shapes ----
    ncmp = np.zeros((N_CORES, NLEV), np.int64)
    for uid, c in comp_core.items():
        ncmp[c, uid // NSPAN] += 1
    NC = [int(_rup(ncmp[:, l].max(), P)) for l in range(NLEV)]

    def widths(n):
        out, off = [], 0
        while off < n:
            w = WTILE if n - off >= WTILE else P
            out.append(w)
            off += w
        return out

    W0 = widths(NC[0])
    W12 = [widths(NC[1]), widths(NC[2])]

    core_rows = [[sorted(uid % NSPAN for uid, cc in comp_core.items()
                         if cc == c and uid // NSPAN == l)
                  for l in range(NLEV)] for c in range(N_CORES)]

    # dedup token list per core: L1/L2-read tokens in first-use order + extra
    core_dtok = []
    for c in range(N_CORES):
        lst, seen = [], set()
        for l in (1, 2):
            for r in core_rows[c][l]:
                for v in comp_reads[l][r]:
                    v = int(v)
                    if 0 <= v < NPOS:
                        t = int(ids[v])
                        if t not in seen:
                            seen.add(t)
                            lst.append(t)
        for t in extra[c]:
            if t not in seen:
                seen.add(t)
                lst.append(t)
        core_dtok.append(lst)

    NDTOKP = _rup(max(len(l) for l in core_dtok), ABATCH)
    NSTREAM = NDTOKP + 4 * NC[0]

    # slot space
    lvl_base = []
    b = 1 + NDTOKP
    for l in range(NLEV):
        lvl_base.append(b)
        b += NC[l]
    nslots = b
    assert nslots < 32768

    tiles = []   # (level, base_slot, W)  for l = 1, 2 only
    for li, l in enumerate((1, 2)):
        off = 0
        for w in W12[li]:
            tiles.append((l, lvl_base[l] + off, w))
            off += w

    inv_vals = set()
    core_rd = []
    core_bounds = []
    core_sl0 = []        # L0 stream content: emb row ids (or -1 = zeros)
    core_cnt0 = []
    core_slot_of_comp = []
    core_tok_slot = []
    for c in range(N_CORES):
        slot_of_tok = {t: 1 + i for i, t in enumerate(core_dtok[c])}
        core_tok_slot.append(slot_of_tok)
        slot_of_comp = {}

        # L0: per-instance stream sections (k-major per tile)
        rows0 = core_rows[c][0]
        for i, r in enumerate(rows0):
            slot_of_comp[0 * NSPAN + int(r)] = lvl_base[0] + i
            inv_vals.add(1.0 / max(int(comp_cnt[0][r]), 1))
        sl0 = np.full(4 * NC[0], -1, np.int64)
        cnt0 = np.zeros(NC[0], np.float32)
        off = 0
        for w in W0:
            for j in range(w):
                i = off + j
                if i < len(rows0):
                    r = rows0[i]
                    cnt0[i] = max(int(comp_cnt[0][r]), 1)
                    for k in range(4):
                        v = int(comp_reads[0][r, k])
                        if v >= 0:
                            assert v < NPOS
                            sl0[4 * off + k * w + j] = int(ids[v])
                else:
                    cnt0[i] = 1.0
            off += w
        core_sl0.append(sl0)
        core_cnt0.append(cnt0)

        def vslot(v):
            v = int(v)
            if v == -1:
                return 0
            if v < NPOS:
                return slot_of_tok[int(ids[v])]
            return slot_of_comp[v - NPOS]

        rd_all, bounds = [], []
        for l in (1, 2):
            rows = core_rows[c][l]

            def row_bound(r):
                return max((vslot(v) for v in comp_reads[l][r]), default=0)
            rows = sorted(rows, key=lambda r: (row_bound(r), r))
            for i, r in enumerate(rows):
                slot_of_comp[l * NSPAN + int(r)] = lvl_base[l] + i
                inv_vals.add(1.0 / max(int(comp_cnt[l][r]), 1))
            rs = np.zeros((NC[l], 4), np.int64)
            for i, r in enumerate(rows):
                for k in range(4):
                    rs[i, k] = vslot(comp_reads[l][r, k])
            off = 0
            for w in ([wd for wd in W12[l - 1]]):
                blk = rs[off:off + w]      # [w, 4]
                # two gathers per tile: k-pair halves, k-major inside
                rd_all.append(blk.T[0:2].reshape(-1))
                rd_all.append(blk.T[2:4].reshape(-1))
                bounds.append(max(1, int(blk.max()) + 1))
                off += w
        core_rd.append(np.concatenate(rd_all))
        core_bounds.append(bounds)
        core_slot_of_comp.append(slot_of_comp)

    bounds = tuple(max(core_bounds[c][i] for c in range(N_CORES))
                   for i in range(len(tiles)))
    for i, (_, tbase, w) in enumerate(tiles):
        assert bounds[i] <= tbase

    if not inv_vals:
        inv_vals = {0.25}
    assert len(inv_vals) == 1, f"non-uniform span counts {inv_vals}"
    inv_uniform = float(inv_vals.pop())

    # ---- output maps ----
    pos_core = np.empty(NPOS, np.int64)
    pos_slot = np.empty(NPOS, np.int64)
    for p in range(NPOS):
        v = int(final_ver[p])
        if v < NPOS:
            t = int(ids[v])
            c = tok_canon[t]
            pos_core[p] = c
            pos_slot[p] = core_tok_slot[c][t]
        else:
            c = comp_core[v - NPOS]
            pos_core[p] = c
            pos_slot[p] = core_slot_of_comp[c][v - NPOS]

    cores = []
    for c in range(N_CORES):
        cores.append(dict(dtok=core_dtok[c], sl0=core_sl0[c],
                          cnt0=core_cnt0[c], rd=core_rd[c]))
    meta = dict(NDTOKP=NDTOKP, NSTREAM=NSTREAM, NC0=NC[0], W0=tuple(W0),
                lvl_base=tuple(lvl_base), tiles=tuple(tiles), bounds=bounds,
                nslots=nslots, inv=inv_uniform,
                pos_core=pos_core, pos_slot=pos_slot)
    return cores, meta


def wrap_idx16(idx):
    """[n] -> [128, n/16] int16 layout for gpsimd gathers (i -> (i%16, i//16))."""
    idx = np.asarray(idx, np.int64)
    n = len(idx)
    assert n % 16 == 0 and idx.max() < 32768 and idx.min() >= 0
    w = idx.reshape(n // 16, 16).T.astype(np.int16)
    return np.tile(w, (8, 1))


# --------------------------------------------------------------------------
# bass program
# --------------------------------------------------------------------------

def build_bass(NDTOKP, NSTREAM, NC0, W0, lvl_base, tiles, bounds, nslots,
               has_bd, has_b1, has_b2, inv):
    nc = bacc.Bacc("TRN2", target_bir_lowering=False, debug=False,
                   num_devices=N_CORES, num_swdge_queues=4)

    QCH = _rup(-(-NSTREAM // 4), ABATCH)
    emb_sT = nc.dram_tensor("emb_sT", [D // P, P, NSTREAM], BF16,
                            kind="ExternalInput")
    w_nat = nc.dram_tensor("w_nat", [P, D // P, CD], BF16,
                           kind="ExternalInput")
    w_spl = nc.dram_tensor("w_spl", [P, D // P, CD // P, P], BF16,
                           kind="ExternalInput")
    b_down = nc.dram_tensor("b_down", [1, CD], F32, kind="ExternalInput")
    wc1T = nc.dram_tensor("wc1T", [P, CD // P, HD // P, P], BF16,
                          kind="ExternalInput")
    bc1e = nc.dram_tensor("bc1e", [1, HD], F32, kind="ExternalInput")
    wc2_n = nc.dram_tensor("wc2_n", [P, HD // P, CD], BF16,
                           kind="ExternalInput")
    bc2 = nc.dram_tensor("bc2", [1, CD], F32, kind="ExternalInput")
    cnt0 = nc.dram_tensor("cnt0", [1, max(NC0, 1)], F32, kind="ExternalInput")
    tot_idx = sum(4 * w for (_, _, w) in tiles)
    rd_idx = nc.dram_tensor("rd_idx", [P, tot_idx // 16], I16,
                            kind="ExternalInput")
    vlog = nc.dram_tensor("vlog", [nslots, CD], BF16, kind="ExternalOutput")

    with tile.TileContext(nc) as tc, ExitStack() as ctx:
        cst = ctx.enter_context(tc.tile_pool(name="cst", bufs=1))
        sb = ctx.enter_context(tc.tile_pool(name="sb", bufs=3))
        ps = ctx.enter_context(tc.tile_pool(name="ps", bufs=2, space="PSUM"))

        rd_sb = cst.tile([P, tot_idx // 16], I16)
        nc.scalar.dma_start(rd_sb[:], rd_idx[:])
        w_sb = cst.tile([P, D // P, CD], BF16)
        nc.scalar.dma_start(w_sb[:], w_nat[:])
        ws_sb = cst.tile([P, D // P, CD // P, P], BF16)
        nc.scalar.dma_start(ws_sb[:], w_spl[:])
        wc1_sb = cst.tile([P, CD // P, HD // P, P], BF16)
        nc.scalar.dma_start(wc1_sb[:], wc1T[:])
        wc2_sb = cst.tile([P, HD // P, CD], BF16)
        nc.scalar.dma_start(wc2_sb[:], wc2_n[:])

        ones1 = cst.tile([1, WTILE], F32)
        nc.vector.memset(ones1[:], 1.0)
        bd_sb = cst.tile([1, CD], F32)
        nc.scalar.dma_start(bd_sb[:], b_down[:])
        bc1_sb = cst.tile([1, HD], F32)
        nc.scalar.dma_start(bc1_sb[:], bc1e[:])
        bc2_sb = cst.tile([1, CD], F32)
        nc.scalar.dma_start(bc2_sb[:], bc2[:])
        cnt0_sb = cst.tile([1, max(NC0, 1)], F32)
        nc.scalar.dma_start(cnt0_sb[:], cnt0[:])

        # zero row (slot 0)
        zrow = cst.tile([1, CD], BF16)
        nc.vector.memset(zrow[:], 0.0)
        nc.scalar.dma_start(vlog[0:1, :], zrow[:])

        # whole pre-transposed stream, SBUF resident
        embT = cst.tile([P, D // P, NSTREAM], BF16)
        q0 = 0
        while q0 < NSTREAM:
            q1 = min(NSTREAM, q0 + QCH)
            for k in range(D // P):
                nc.sync.dma_start(embT[:, k, q0:q1], emb_sT[k, :, q0:q1])
            q0 = q1

        qn = [0]

        def next_q():
            q = qn[0] % 4
            qn[0] += 1
            return q

        # ---- A phase: deduped tokens, batched ----
        for b in range(NDTOKP // ABATCH):
            stg = sb.tile([P, ABATCH // P, CD], BF16, tag="stg", bufs=3)
            for t in range(ABATCH // P):
                r0 = b * ABATCH + t * P
                acc = ps.tile([P, CD], F32, tag="acc", bufs=2)
                if has_bd:
                    nc.tensor.matmul(acc[:], lhsT=ones1[:, 0:P],
                                     rhs=bd_sb[:], start=True, stop=False)
                for k in range(D // P):
                    nc.tensor.matmul(acc[:], lhsT=embT[:, k, r0:r0 + P],
                                     rhs=w_sb[:, k, :],
                                     start=(k == 0 and not has_bd),
                                     stop=(k == D // P - 1))
                nc.vector.tensor_copy(out=stg[:, t, :], in_=acc[:])
            dst = vlog[1 + b * ABATCH:1 + (b + 1) * ABATCH, :]
            nc.scalar.dma_start(dst.rearrange("(t p) d -> p t d", p=P),
                                stg[:])

        def mlp_and_store(meanT, tbase, w):
            """meanT [P, 2, w] bf16 (unscaled sum); writes vlog rows."""
            hT = sb.tile([P, HD // P, w], BF16, tag=f"hT{w}", bufs=2)
            for i2 in range(0, HD // P, 2):
                phb = ps.tile([P, 2 * WTILE], F32, tag="ph", bufs=2)
                for di in range(2):
                    ph = phb[:, di * w:(di + 1) * w]
                    i = i2 + di
                    if has_b1:
                        nc.tensor.matmul(ph, lhsT=bc1_sb[:, i * P:(i + 1) * P],
                                         rhs=ones1[:, 0:w],
                                         start=True, stop=False)
                    for k in range(CD // P):
                        nc.tensor.matmul(ph, lhsT=wc1_sb[:, k, i, :],
                                         rhs=meanT[:, k, :],
                                         start=(k == 0 and not has_b1),
                                         stop=(k == CD // P - 1))
                nc.scalar.activation(
                    out=hT[:, i2:i2 + 2, :], in_=phb[:, 0:2 * w],
                    func=mybir.ActivationFunctionType.Gelu_apprx_tanh,
                    scale=float(inv))
            pstg = sb.tile([P, w // P, CD], BF16, tag=f"pstg{w}", bufs=3)
            for h in range(w // P):
                po = ps.tile([P, CD], F32, tag="po", bufs=2)
                if has_b2:
                    nc.tensor.matmul(po[:], lhsT=ones1[:, 0:P],
                                     rhs=bc2_sb[:], start=True, stop=False)
                for k in range(HD // P):
                    nc.tensor.matmul(po[:],
                                     lhsT=hT[:, k, h * P:(h + 1) * P],
                                     rhs=wc2_sb[:, k, :],
                                     start=(k == 0 and not has_b2),
                                     stop=(k == HD // P - 1))
                nc.vector.tensor_copy(out=pstg[:, h, :], in_=po[:])
            dst = vlog[tbase:tbase + w, :]
            nc.scalar.dma_start(dst.rearrange("(t p) d -> p t d", p=P),
                                pstg[:])

        # ---- L0 tiles: fused mean-downprojection ----
        off = 0
        for w in W0:
            sec = NDTOKP + 4 * off
            meanT = sb.tile([P, 2, w], BF16, tag=f"meanT{w}")
            for j in range(CD // P):
                mp = ps.tile([P, WTILE], F32, tag="mp", bufs=2)
                m = mp[:, 0:w]
                if has_bd:
                    nc.tensor.matmul(m, lhsT=bd_sb[:, j * P:(j + 1) * P],
                                     rhs=cnt0_sb[:, off:off + w],
                                     start=True, stop=False)
                nmm = 4 * (D // P)
                i = 0
                for k in range(4):
                    for kc in range(D // P):
                        s0 = sec + k * w
                        nc.tensor.matmul(
                            m, lhsT=ws_sb[:, kc, j, :],
                            rhs=embT[:, kc, s0:s0 + w],
                            start=(i == 0 and not has_bd),
                            stop=(i == nmm - 1))
                        i += 1
                nc.vector.tensor_copy(out=meanT[:, j, :], in_=m)
            mlp_and_store(meanT, lvl_base[0] + off, w)
            off += w

        # ---- L1/L2 tiles: gathered operands ----
        idx_off = 0
        for ti, (l, tbase, w) in enumerate(tiles):
            bound = bounds[ti]
            meanT = sb.tile([P, 2, w], BF16, tag=f"meanT{w}")
            half = []
            for h in range(2):
                g = sb.tile([P, 2, 2 * w], BF16, tag=f"g{w}_{h}", bufs=2)
                nc.gpsimd.dma_gather(
                    g[:], vlog[0:bound, :],
                    rd_sb[:, idx_off:idx_off + 2 * w // 16],
                    2 * w, 2 * w, CD, transpose=True, queue_num=next_q())
                idx_off += 2 * w // 16
                s = sb.tile([P, 2, w], F32, tag=f"s{w}_{h}")
                nc.vector.tensor_add(out=s[:], in0=g[:, :, 0:w],
                                     in1=g[:, :, w:2 * w])
                half.append(s)
            nc.vector.tensor_add(out=meanT[:], in0=half[0][:], in1=half[1][:])
            mlp_and_store(meanT, tbase, w)

    nc.compile()
    return nc


_CACHE = {}


def _get_bass(key):
    if key not in _CACHE:
        _CACHE[key] = build_bass(*key)
    return _CACHE[key]


def _install_ntff_hook():
    try:
        import antenv.axon_hooks  # noqa: F401
        return
    except ImportError:
        pass
    try:
        import trn_agent_boot.trn_boot as _tb
        hooks = types.ModuleType('antenv.axon_hooks')
        hook = _tb._ntff_profile_via_ctypes('/opt/axon/libaxon_pjrt.so')
        hooks.get_axon_ntff_profile_hook = lambda: hook
        hooks.set_axon_ntff_profile_hook = lambda h: None
        sys.modules['antenv.axon_hooks'] = hooks
    except Exception:
        pass


def run(inputs, trace=False):
    """Returns (full_output, exec_time_ns or None)."""
    inp = {k: (np.asarray(v) if hasattr(v, 'shape') else v)
           for k, v in inputs.items()}
    spans_list = [inp["spans0"], inp["spans1"], inp["spans2"]]
    cores, meta = plan(inp["chunk_input_ids"], spans_list)

    def f32(x):
        return np.ascontiguousarray(x, np.float32)

    b_down = f32(inp["b_down"]).reshape(1, CD)
    bc1 = f32(inp["bc1"]).reshape(1, HD)
    bc2 = f32(inp["bc2"]).reshape(1, CD)
    has_bd = bool(np.any(b_down))
    has_b1 = bool(np.any(bc1))
    has_b2 = bool(np.any(bc2))

    nc = _get_bass((meta["NDTOKP"], meta["NSTREAM"], meta["NC0"], meta["W0"],
                    meta["lvl_base"], meta["tiles"], meta["bounds"],
                    meta["nslots"], has_bd, has_b1, has_b2, meta["inv"]))

    emb_bf = np.asarray(inp["emb_table"], np.float32).astype(ml_dtypes.bfloat16)

    def bf16(x):
        return np.ascontiguousarray(
            np.asarray(x, np.float32).astype(ml_dtypes.bfloat16))

    w_bf = bf16(inp["w_down"])
    shared = dict(
        w_nat=np.ascontiguousarray(w_bf.reshape(6, P, CD)
                                   .transpose(1, 0, 2)),
        w_spl=np.ascontiguousarray(w_bf.reshape(6, P, 2, P)
                                   .transpose(1, 0, 2, 3)),
        b_down=b_down,
        wc1T=np.ascontiguousarray(bf16(inp["wc1"]).reshape(2, P, 8, P)
                                  .transpose(1, 0, 2, 3)),
        bc1e=np.ascontiguousarray(bc1 / meta["inv"]),
        wc2_n=np.ascontiguousarray(bf16(inp["wc2"]).reshape(8, P, CD)
                                   .transpose(1, 0, 2)),
        bc2=bc2,
    )
    NDTOKP, NSTREAM = meta["NDTOKP"], meta["NSTREAM"]
    in_maps = []
    for c in range(N_CORES):
        core = cores[c]
        m = dict(shared)
        stream = np.zeros((NSTREAM, D), ml_dtypes.bfloat16)
        dt = core["dtok"]
        if len(dt):
            stream[:len(dt)] = emb_bf[np.asarray(dt, np.int64)]
        sl0 = core["sl0"]
        live = sl0 >= 0
        if live.any():
            stream[NDTOKP + np.nonzero(live)[0]] = emb_bf[sl0[live]]
        m["emb_sT"] = np.ascontiguousarray(
            stream.reshape(NSTREAM, 6, P).transpose(1, 2, 0))
        m["cnt0"] = core["cnt0"].reshape(1, -1)
        m["rd_idx"] = wrap_idx16(core["rd"])
        in_maps.append(m)

    _install_ntff_hook()
    res = run_bass_kernel_spmd(nc, in_maps, core_ids=list(range(N_CORES)),
                               trace=trace)
    vals = np.stack([np.asarray(res.results[c]["vlog"]).astype(np.float32)
                     for c in range(N_CORES)])     # [8, nslots, 256]
    full = vals[meta["pos_core"], meta["pos_slot"]]
    return full.reshape(16, 2048, CD), res.exec_time_ns


def kernel(**inputs):
    out, _ = run(inputs, trace=False)
    return out
